# revision 4
# baseline (speedup 1.0000x reference)
"""GQA attention (B=1, S=2048, H=2048, 32 q-heads / 8 kv-heads, hd=64)
on 8 Trainium2 NeuronCores.

Sharding: tensor-parallel over heads for QKV+attention (core c owns
q-heads 4c..4c+3 and kv-head c), then sequence-parallel o_proj via
three AllToAll rounds of decreasing size (q [0,1024) / [1024,1792) /
[1792,2048)) so the last collective + o_proj tail is small. Per round,
core c owns qn={128,96,32} query rows; each core holds the FULL wo
(bf16) and computes its rows locally. Round 2 (M=32) is computed with
4-way PE column tiling + a transpose-reduce matmul, output transposed
(out_r2T), host fixes it up.

All large matmuls are bf16. Engine queues are FIFO in emission order.
B-phase (per-head RMSNorm+rope) activations are grouped (all Ln, then
all Exp) so the ACT table is not thrashed against the attention Exp
stream; 1/l uses the fast DVE reciprocal. A tiny AllToAll early
prewarms the collective path so round-0's A2A is not slowed by
first-call setup.
"""
import numpy as np
import sys

sys.path.insert(0, "/opt/trn_rl_repo")

import concourse.bacc as bacc  # noqa: E402
import concourse.mybir as mybir  # noqa: E402
import concourse.tile as tile  # noqa: E402
from concourse import bass_utils  # noqa: E402

f32 = mybir.dt.float32
bf16 = mybir.dt.bfloat16
AF = mybir.ActivationFunctionType
BF16NP = mybir.dt.np(bf16)

N_CORES = 8
S = 2048
HID = 2048
HD = 64
ROPE_THETA = 10000.0
RMS_EPS = 1e-6
SCALING = HD ** -0.5              # 0.125
NK = HID // 128                   # 16 contraction tiles
# rounds: (q_start, per-core qpos)
ROUNDS = ((0, 128), (1024, 96), (1792, 32))

_NC_CACHE = None
LAST_RESULTS = None


def _build():
    nc = bacc.Bacc("TRN2", target_bir_lowering=False, debug=False,
                   num_devices=N_CORES)

    def din(name, shape, dt):
        return nc.dram_tensor(name, shape, dt, kind="ExternalInput").ap()

    xP = din("xP", [2 * NK * 128, 1024], bf16)
    wq0 = din("wq0", [128, HID], bf16)
    wq1 = din("wq1", [128, HID], bf16)
    wkv = din("wkv", [128, HID], bf16)     # [wv | wk] columns pretiled
    wof = din("wof", [128, NK * HID], bf16)  # FULL wo, pretiled
    cos2 = din("cos2", [128, S], bf16)
    ss2 = din("ss2", [128, S], bf16)
    ew_q = din("ew_q", [2, 128], bf16)
    ew_k = din("ew_k", [2, 128], bf16)
    e2 = din("e2", [2, 128], bf16)
    e2t = din("e2t", [128, 2], bf16)
    mask = din("mask", [128, 128], bf16)
    ident = din("ident", [64, 64], bf16)
    r4 = din("r4", [128, 32], bf16)        # R[32g+j, j] = 1

    out_rs = nc.dram_tensor("out_rs", [224, S], f32,
                            kind="ExternalOutput").ap()
    out_r2T = nc.dram_tensor("out_r2T", [HID, 32], f32,
                             kind="ExternalOutput").ap()

    with tile.TileContext(nc) as tc:
        with tc.tile_pool(name="consts", bufs=1) as cp, \
             tc.tile_pool(name="dram", bufs=1, space="DRAM") as dp:
            c_wq0 = cp.tile([128, HID], bf16, tag="w")
            c_wq1 = cp.tile([128, HID], bf16, tag="w2")
            c_wkv = cp.tile([128, HID], bf16, tag="w3")
            c_wo = cp.tile([128, NK * HID], bf16, tag="w4")
            c_cos = cp.tile([128, S], bf16, tag="c1")
            c_ss = cp.tile([128, S], bf16, tag="c2")
            c_ewq = cp.tile([2, 128], bf16, tag="c3")
            c_ewk = cp.tile([2, 128], bf16, tag="c4")
            c_e2 = cp.tile([2, 128], bf16, tag="c5")
            c_e2t = cp.tile([128, 2], bf16, tag="c5t")
            c_mask = cp.tile([128, 128], bf16, tag="c6")
            c_id = cp.tile([64, 64], bf16, tag="c7")
            c_r4 = cp.tile([128, 32], bf16, tag="c7r")
            c_eps = cp.tile([2, 1], f32, tag="c8")
            c_scr = cp.tile([128, 640], bf16, tag="c9")

            nc.vector.memset(c_scr[:], 0.0)
            nc.vector.memset(c_eps[:], RMS_EPS)

            pre_in = dp.tile([8, 16], bf16, name="prei", tag="pi")
            pre_out = dp.tile([8, 16], bf16, name="preo", tag="po")
            a2a_in = [dp.tile([S, qn], bf16, name=f"a2ai{r}",
                              tag=f"ai{r}")
                      for r, (_, qn) in enumerate(ROUNDS)]
            a2a_out = [dp.tile([S, qn], bf16, name=f"a2ao{r}",
                               tag=f"ao{r}")
                       for r, (_, qn) in enumerate(ROUNDS)]

            # prewarm the collective path with a tiny AllToAll
            nc.sync.dma_start(pre_in[:], c_scr[0:8, 0:16])
            nc.gpsimd.collective_compute(
                "AllToAll", mybir.AluOpType.bypass,
                replica_groups=[list(range(N_CORES))],
                ins=[pre_in[:, :].opt()], outs=[pre_out[:, :].opt()])

            # weight loads: whole-tensor contiguous, one per queue
            nc.sync.dma_start(c_wq0[:], wq0)
            nc.scalar.dma_start(c_e2t[:], e2t)
            nc.scalar.dma_start(c_ewq[:], ew_q)
            nc.scalar.dma_start(c_ewk[:], ew_k)
            nc.scalar.dma_start(c_id[:], ident)
            nc.scalar.dma_start(c_e2[:], e2)
            nc.scalar.dma_start(c_r4[:], r4)
            nc.scalar.dma_start(c_mask[:], mask)
            nc.scalar.dma_start(c_wq1[:], wq1)
            nc.gpsimd.dma_start(c_wkv[:], wkv)

            qkv = {
                "q0": cp.tile([128, S], bf16, tag="q0", name="q0"),
                "q1": cp.tile([128, S], bf16, tag="q1", name="q1"),
                "kv": cp.tile([128, S], bf16, tag="kv", name="kv"),
            }
            qr0h = [cp.tile([128, 1024], bf16, tag=f"qr0{h}",
                            name=f"qr0{h}") for h in range(2)]
            qr1h = [cp.tile([128, 1024], bf16, tag=f"qr1{h}",
                            name=f"qr1{h}") for h in range(2)]
            krdh = [cp.tile([128, 1024], bf16, tag=f"krd{h}",
                            name=f"krd{h}") for h in range(2)]
            vah = [cp.tile([128, 8 * (HD + 1)], bf16, tag=f"va{h}",
                           name=f"va{h}") for h in range(2)]
            attn_bf = [cp.tile([128, S], bf16, tag=f"abf{i}",
                               name=f"abf{i}") for i in range(2)]

            with tc.tile_pool(name="xt", bufs=4) as xp, \
                 tc.tile_pool(name="sbB", bufs=2) as sbB:

                def phase_a(qh, psA):
                    hs = slice(1024 * qh, 1024 * qh + 1024)
                    pq = [psA.tile([128, 1024], f32, tag="pa",
                                   name=f"pa{qh}_{j}") for j in range(3)]
                    for t in range(NK):
                        xt = xp.tile([128, 1024], bf16, tag="xt")
                        eng = (nc.sync, nc.scalar, nc.gpsimd)[t % 3]
                        xr = (qh * NK + t) * 128
                        eng.dma_start(xt[:], xP[xr:xr + 128, :])
                        st = (t == 0)
                        sp = (t == NK - 1)
                        tc_ = slice(128 * t, 128 * (t + 1))
                        for j, w in ((0, c_wq0), (1, c_wq1), (2, c_wkv)):
                            nc.tensor.matmul(pq[j][:, 0:512], w[:, tc_],
                                             xt[:, 0:512],
                                             start=st, stop=sp)
                            nc.tensor.matmul(pq[j][:, 512:1024],
                                             w[:, tc_], xt[:, 512:1024],
                                             start=st, stop=sp)
                    for j, key in ((0, "q0"), (1, "q1"), (2, "kv")):
                        nc.vector.tensor_copy(qkv[key][:, hs], pq[j][:])

                # --- B-phase, split so ACT ops group by table-set ---
                def b_p1a(qh, si, key, is_kv, psP, ptag):
                    hs = slice(1024 * qh, 1024 * qh + 1024)
                    src = qkv[key]
                    if is_kv:
                        nc.gpsimd.memset(vah[qh][:], 1.0)
                        for lt in range(8):
                            ptr = psP.tile([128, 64], bf16, tag=ptag,
                                           name=f"ptr{qh}_{lt}")
                            nc.tensor.transpose(
                                ptr[:],
                                src[0:64, 1024 * qh + 128 * lt:
                                    1024 * qh + 128 * (lt + 1)],
                                c_id[:])
                            nc.vector.tensor_copy(
                                vah[qh][:, (HD + 1) * lt:
                                        (HD + 1) * lt + HD],
                                ptr[:])
                    sq = sbB.tile([128, 1024], bf16, tag="sq",
                                  bufs=2, name=f"sq{qh}_{si}")
                    nc.vector.tensor_mul(sq[:], src[:, hs], src[:, hs])
                    lnvs = {}
                    for u in range(2):
                        us = slice(512 * u, 512 * u + 512)
                        pss = psP.tile([2, 512], f32, tag=ptag,
                                       name=f"ss{qh}_{si}_{u}")
                        nc.tensor.matmul(pss[:], c_e2t[:], sq[:, us],
                                         start=True, stop=True)
                        lnv = sbB.tile([2, 512], bf16, tag="lnv",
                                       bufs=6, name=f"lnv{qh}{si}{u}")
                        nc.scalar.activation(lnv[:], pss[:], AF.Ln,
                                             scale=1.0 / HD,
                                             bias=c_eps[:])
                        lnvs[u] = lnv
                    return lnvs

                def b_p1b(qh, si, lnvs):
                    rstds = {}
                    for u in range(2):
                        rr = sbB.tile([2, 512], bf16, tag="rstdr",
                                      bufs=6, name=f"rr{qh}{si}{u}")
                        nc.scalar.activation(rr[:], lnvs[u][:],
                                             AF.Exp, scale=-0.5)
                        rstds[u] = rr
                    return rstds

                def b_p2(qh, si, key, ew, dst, is_kv, rstds, psP, ptag):
                    hs = slice(1024 * qh, 1024 * qh + 1024)
                    src = qkv[key]
                    rows = slice(64, 128) if is_kv else slice(0, 128)
                    nrm = sbB.tile([128, 1024], f32, tag="nrm",
                                   bufs=2, name=f"nrm{qh}_{si}")
                    for u in range(2):
                        cs = slice(1024 * qh + 512 * u,
                                   1024 * qh + 512 * u + 512)
                        us = slice(512 * u, 512 * u + 512)
                        pb = psP.tile([128, 512], f32, tag=ptag,
                                      name=f"pb{qh}_{si}_{u}")
                        nc.tensor.matmul(pb[:], ew[:], rstds[u][:],
                                         start=True, stop=True)
                        nc.vector.tensor_mul(nrm[rows, us],
                                             src[rows, cs], pb[rows, :])
                    sh = sbB.tile([128, 1024], f32, tag="sh",
                                  bufs=2, name=f"sh{qh}_{si}")
                    if is_kv:
                        nc.sync.dma_start(sh[64:96, :], nrm[96:128, :])
                        nc.sync.dma_start(sh[96:128, :], nrm[64:96, :])
                    else:
                        nc.sync.dma_start(sh[0:32, :], nrm[32:64, :])
                        nc.sync.dma_start(sh[32:64, :], nrm[0:32, :])
                        nc.sync.dma_start(sh[64:96, :], nrm[96:128, :])
                        nc.sync.dma_start(sh[96:128, :], nrm[64:96, :])
                    t2 = sbB.tile([128, 1024], f32, tag="t2",
                                  bufs=1, name=f"t2{qh}_{si}")
                    nc.vector.tensor_mul(t2[rows, :], sh[rows, :],
                                         c_ss[rows, slice(1024 * qh,
                                                          1024 * qh
                                                          + 1024)])
                    t1 = sbB.tile([128, 1024], f32, tag="sh",
                                  bufs=2, name=f"t1{qh}_{si}")
                    nc.vector.tensor_mul(t1[rows, :], nrm[rows, :],
                                         c_cos[rows, hs])
                    nc.vector.tensor_add(dst[rows, :], t1[rows, :],
                                         t2[rows, :])
                    if is_kv:
                        nc.sync.dma_start(dst[0:64, :], dst[64:128, :])

                B0 = (("kv", c_ewk, True), ("q0", c_ewq, False),
                      ("q1", c_ewq, False))

                def b_group(qh, dsts, psP, ptag):
                    ls = [b_p1a(qh, si, key, ik, psP, ptag)
                          for si, (key, _, ik) in enumerate(B0)]
                    rs = [b_p1b(qh, si, ls[si]) for si in range(3)]
                    for si, (key, ew, ik) in enumerate(B0):
                        b_p2(qh, si, key, ew, dsts[si], ik, rs[si],
                             psP, ptag)

                # ---- scope 1: warmup + A0 + B0-group + A1 ----
                with tc.tile_pool(name="psA", bufs=3,
                                  space="PSUM") as psA, \
                     tc.tile_pool(name="psM", bufs=2,
                                  space="PSUM") as psM:
                    pwm = psM.tile([128, 512], f32, tag="m", name="pwm")
                    for i in range(32):
                        nc.tensor.matmul(pwm[:], c_scr[:, 0:128],
                                         c_scr[:, 128:640],
                                         start=True, stop=True)
                    phase_a(0, psA)
                    nc.gpsimd.dma_start(c_cos[:], cos2)
                    nc.gpsimd.dma_start(c_ss[:], ss2)
                    b_group(0, (krdh[0], qr0h[0], qr1h[0]), psM, "m")
                    phase_a(1, psA)
                    for h in range(8):
                        cs_ = slice(4096 * h, 4096 * (h + 1))
                        nc.gpsimd.dma_start(c_wo[:, cs_], wof[:, cs_])

                # ---- scope 2: B1 + qchunks + A2As + o_proj ----
                with tc.tile_pool(name="sbC", bufs=4) as sbC, \
                     tc.tile_pool(name="atk", bufs=2) as akp, \
                     tc.tile_pool(name="psS", bufs=2,
                                  space="PSUM") as psS, \
                     tc.tile_pool(name="psPV", bufs=2,
                                  space="PSUM") as psPV, \
                     tc.tile_pool(name="psB", bufs=1,
                                  space="PSUM") as psB, \
                     tc.tile_pool(name="psO", bufs=1,
                                  space="PSUM") as psO:

                    def qchunk(q0, W):
                        qs = slice(q0, q0 + W)
                        qhh = q0 // 1024
                        qcol0 = q0 - 1024 * qhh
                        ntile = (q0 + W) // 128
                        t0k = q0 // 128
                        rnd = 0 if q0 < 1024 else (1 if q0 < 1792
                                                   else 2)
                        rs_, qn = ROUNDS[rnd]
                        for hp, qrh in ((0, qr0h), (1, qr1h)):
                            qr = qrh[qhh]
                            ppv_a = psPV.tile([65, W], f32, tag="pv",
                                              name=f"pva{q0}_{hp}")
                            ppv_b = psPV.tile([65, W], f32, tag="pv",
                                              name=f"pvb{q0}_{hp}")
                            for t in range(ntile):
                                r = t - t0k
                                off = max(0, r) * 128
                                qlo = qcol0 + off
                                qlen = W - off
                                kh = t // 8
                                krd = krdh[kh]
                                v_aug = vah[kh]
                                tl = t - 8 * kh
                                kc = slice(128 * tl, 128 * (tl + 1))
                                vs = slice((HD + 1) * tl,
                                           (HD + 1) * tl + HD + 1)
                                st = (t == 0)
                                sp = (t == ntile - 1)
                                ps_s = psS.tile([128, 1024], f32,
                                                tag="s")
                                nc.tensor.matmul(
                                    ps_s[:, 0:qlen], krd[0:64, kc],
                                    qr[0:64, qlo:qlo + qlen],
                                    start=True, stop=True)
                                nc.tensor.matmul(
                                    ps_s[:, 512:512 + qlen],
                                    krd[64:128, kc],
                                    qr[64:128, qlo:qlo + qlen],
                                    start=True, stop=True)
                                pt = sbC.tile([128, 1024], bf16,
                                              tag="pt")
                                nc.scalar.activation(
                                    pt[:, 0:512 + qlen],
                                    ps_s[:, 0:512 + qlen],
                                    AF.Exp, scale=SCALING)
                                if r >= 0:
                                    nc.vector.tensor_mul(
                                        pt[:, 0:128], pt[:, 0:128],
                                        c_mask[:])
                                    nc.vector.tensor_mul(
                                        pt[:, 512:640], pt[:, 512:640],
                                        c_mask[:])
                                nc.tensor.matmul(
                                    ppv_a[:, off:W], v_aug[:, vs],
                                    pt[:, 0:qlen], start=st, stop=sp)
                                nc.tensor.matmul(
                                    ppv_b[:, off:W], v_aug[:, vs],
                                    pt[:, 512:512 + qlen],
                                    start=st, stop=sp)
                            # normalize + stage
                            nc.vector.tensor_copy(
                                attn_bf[hp][0:64, qs], ppv_a[0:64, :])
                            stgb = sbC.tile([64, W], bf16, tag="stg",
                                            bufs=2,
                                            name=f"sg{q0}_{hp}")
                            nc.vector.tensor_copy(stgb[:],
                                                  ppv_b[0:64, :])
                            nc.sync.dma_start(
                                attn_bf[hp][64:128, qs], stgb[:])
                            la = sbC.tile([65, W], f32, tag="la",
                                          bufs=2, name=f"la{q0}_{hp}")
                            nc.vector.tensor_copy(la[64:65, :],
                                                  ppv_a[64:65, :])
                            lb = sbC.tile([65, W], f32, tag="la",
                                          bufs=2, name=f"lb{q0}_{hp}")
                            nc.vector.tensor_copy(lb[64:65, :],
                                                  ppv_b[64:65, :])
                            lf = sbC.tile([2, W], f32, tag="lf",
                                          bufs=2, name=f"lf{q0}_{hp}")
                            nc.sync.dma_start(lf[0:1, :], la[64:65, :])
                            nc.gpsimd.dma_start(lf[1:2, :],
                                                lb[64:65, :])
                            rlf = sbC.tile([2, W], f32, tag="lf",
                                           bufs=2, name=f"rf{q0}_{hp}")
                            nc.vector.reciprocal_approx_fast(rlf[:],
                                                             lf[:])
                            rlb = sbC.tile([2, W], bf16, tag="rlb",
                                           bufs=2, name=f"rb{q0}_{hp}")
                            nc.vector.tensor_copy(rlb[:], rlf[:])
                            pb = psB.tile([128, W], f32, tag="b",
                                          name=f"qpb{q0}_{hp}")
                            nc.tensor.matmul(pb[:], c_e2[:], rlb[:],
                                             start=True, stop=True)
                            nc.vector.tensor_mul(
                                attn_bf[hp][:, qs],
                                attn_bf[hp][:, qs], pb[:])
                            seng = nc.gpsimd if hp == 0 else nc.sync
                            for c in range(N_CORES):
                                a = max(q0, rs_ + qn * c)
                                b = min(q0 + W, rs_ + qn * (c + 1))
                                if a < b:
                                    rr_ = 256 * c + 128 * hp
                                    seng.dma_start(
                                        a2a_in[rnd][rr_:rr_ + 128,
                                                    a - rs_ - qn * c:
                                                    b - rs_ - qn * c],
                                        attn_bf[hp][:, a:b])

                    def a2a(rnd):
                        nc.gpsimd.collective_compute(
                            "AllToAll",
                            mybir.AluOpType.bypass,
                            replica_groups=[list(range(N_CORES))],
                            ins=[a2a_in[rnd][:, :].opt()],
                            outs=[a2a_out[rnd][:, :].opt()],
                        )

                    def oproj(rnd):
                        qn = ROUNDS[rnd][1]
                        rb = (0, 128, 0)[rnd]
                        attk = akp.tile([128, NK * qn], bf16,
                                        tag="atk")
                        for kk in range(NK):
                            eng = nc.sync if kk % 2 == 0 else nc.scalar
                            eng.dma_start(
                                attk[:, qn * kk:qn * (kk + 1)],
                                a2a_out[rnd][128 * kk:
                                             128 * (kk + 1), :])
                        pos = psO.tile([128, 512], f32, tag="o",
                                       name=f"pos{rnd}")
                        for sub in range(8):
                            h = 256 * (sub % 2)
                            wcs = 256 * sub
                            if rnd < 2:
                                for kk in range(NK):
                                    nc.tensor.matmul(
                                        pos[0:qn, h:h + 256],
                                        attk[:, qn * kk:qn * (kk + 1)],
                                        c_wo[:, HID * kk + wcs:
                                             HID * kk + wcs + 256],
                                        start=(kk == 0),
                                        stop=(kk == NK - 1))
                                ost = sbC.tile([qn, 256], f32,
                                               tag="ost", bufs=3,
                                               name=f"ost{rnd}_{sub}")
                                nc.vector.tensor_copy(
                                    ost[:], pos[0:qn, h:h + 256])
                                eng = nc.sync if sub % 2 == 0 else \
                                    nc.scalar
                                eng.dma_start(
                                    out_rs[rb:rb + qn,
                                           wcs:wcs + 256], ost[:])
                            else:
                                for g in range(4):
                                    for i in range(4):
                                        kk = 4 * g + i
                                        nc.tensor.matmul(
                                            pos[32 * g:32 * g + 32,
                                                h:h + 256],
                                            attk[:, 32 * kk:
                                                 32 * kk + 32],
                                            c_wo[:, HID * kk + wcs:
                                                 HID * kk + wcs + 256],
                                            start=(i == 0),
                                            stop=(i == 3),
                                            tile_position=(0, 32 * g))
                                ost4 = sbC.tile([128, 256], bf16,
                                                tag="ost", bufs=3,
                                                name=f"o4_{sub}")
                                nc.vector.tensor_copy(ost4[:],
                                                      pos[:,
                                                          h:h + 256])
                                pr = psB.tile([128, 64], f32, tag="b",
                                              name=f"pr{sub}")
                                nc.tensor.matmul(pr[:, 0:32],
                                                 ost4[:, 0:128],
                                                 c_r4[:],
                                                 start=True, stop=True)
                                nc.tensor.matmul(pr[:, 32:64],
                                                 ost4[:, 128:256],
                                                 c_r4[:],
                                                 start=True, stop=True)
                                o2 = sbC.tile([128, 64], f32,
                                              tag="ost", bufs=3,
                                              name=f"o2_{sub}")
                                nc.vector.tensor_copy(o2[:], pr[:])
                                e1 = nc.sync if sub % 2 == 0 else \
                                    nc.scalar
                                e2_ = nc.scalar if sub % 2 == 0 else \
                                    nc.sync
                                e1.dma_start(
                                    out_r2T[wcs:wcs + 128, :],
                                    o2[:, 0:32])
                                e2_.dma_start(
                                    out_r2T[wcs + 128:wcs + 256, :],
                                    o2[:, 32:64])

                    qchunk(0, 512)
                    b_group(1, (krdh[1], qr0h[1], qr1h[1]), psS, "s")
                    qchunk(512, 512)
                    a2a(0)
                    qchunk(1024, 512)
                    qchunk(1536, 256)
                    a2a(1)
                    oproj(0)
                    qchunk(1792, 256)
                    a2a(2)
                    oproj(1)
                    oproj(2)

    nc.compile()
    return nc


def _host_prep(hidden_states, position_ids, wq, wk, wv, wo, q_ln_w, k_ln_w):
    x = np.asarray(hidden_states, dtype=np.float32)[0]        # [S, HID]
    xT = np.ascontiguousarray(x.T).astype(BF16NP)             # [HID, S]
    xP = np.ascontiguousarray(
        xT.reshape(NK, 128, 2, 1024).transpose(2, 0, 1, 3)
    ).reshape(2 * NK * 128, 1024)
    pos = np.asarray(position_ids)[0].astype(np.float32)      # [S]
    inv = 1.0 / (ROPE_THETA ** (np.arange(0, HD, 2, dtype=np.float32) / HD))
    ang = pos[:, None] * inv[None, :]                         # [S, 32]
    emb = np.concatenate([ang, ang], axis=1)                  # [S, 64]
    cosT = np.cos(emb).T.astype(np.float32)                   # [64, S]
    sinT = np.sin(emb).T.astype(np.float32)
    ss = sinT.copy()
    ss[0:32] = -sinT[0:32]
    cos2 = np.tile(cosT, (2, 1))
    ss2 = np.tile(ss, (2, 1))

    e2 = np.zeros((2, 128), dtype=np.float32)
    e2[0, 0:64] = 1.0
    e2[1, 64:128] = 1.0
    ew_q = np.zeros((2, 128), dtype=np.float32)
    ew_q[0, 0:64] = q_ln_w
    ew_q[1, 64:128] = q_ln_w
    ew_k = np.zeros((2, 128), dtype=np.float32)
    ew_k[1, 64:128] = k_ln_w
    msk = (np.arange(128)[:, None] <= np.arange(128)[None, :]) \
        .astype(np.float32)
    ident = np.eye(64, dtype=np.float32)
    r4 = np.zeros((128, 32), dtype=np.float32)
    for g in range(4):
        r4[32 * g + np.arange(32), np.arange(32)] = 1.0

    wq_ = np.asarray(wq, dtype=np.float32)
    wk_ = np.asarray(wk, dtype=np.float32)
    wv_ = np.asarray(wv, dtype=np.float32)
    wo_ = np.asarray(wo, dtype=np.float32)

    def pretile(w):  # [HID, N] -> [128, NK*N] ktile-blocked
        n = w.shape[1]
        return np.ascontiguousarray(
            w.reshape(NK, 128, n).transpose(1, 0, 2).reshape(128, NK * n))

    wof = pretile(wo_).astype(BF16NP)

    in_maps = []
    for c in range(N_CORES):
        qcols = slice(256 * c, 256 * (c + 1))
        kvcols = slice(64 * c, 64 * (c + 1))
        wq_c = np.ascontiguousarray(wq_[:, qcols])
        wkv_c = np.concatenate([wv_[:, kvcols], wk_[:, kvcols]], axis=1)
        in_maps.append({
            "xP": xP,
            "wq0": pretile(wq_c[:, 0:128]).astype(BF16NP),
            "wq1": pretile(wq_c[:, 128:256]).astype(BF16NP),
            "wkv": pretile(wkv_c).astype(BF16NP),
            "wof": wof,
            "cos2": cos2.astype(BF16NP),
            "ss2": ss2.astype(BF16NP),
            "ew_q": ew_q.astype(BF16NP),
            "ew_k": ew_k.astype(BF16NP),
            "e2": e2.astype(BF16NP),
            "e2t": np.ascontiguousarray(e2.T).astype(BF16NP),
            "mask": msk.astype(BF16NP),
            "ident": ident.astype(BF16NP),
            "r4": r4.astype(BF16NP),
        })
    return in_maps


def kernel(hidden_states, position_ids, wq, wk, wv, wo, q_ln_w, k_ln_w):
    global _NC_CACHE, LAST_RESULTS
    if _NC_CACHE is None:
        _NC_CACHE = _build()
    nc = _NC_CACHE
    in_maps = _host_prep(hidden_states, position_ids, wq, wk, wv, wo,
                         q_ln_w, k_ln_w)
    res = bass_utils.run_bass_kernel_spmd(
        nc, in_maps, core_ids=list(range(N_CORES)))
    LAST_RESULTS = res
    out = np.empty((S, HID), dtype=np.float32)
    for c in range(N_CORES):
        o_c = res.results[c]["out_rs"]        # [224, 2048]
        out[128 * c:128 * c + 128, :] = o_c[0:128, :]
        out[1024 + 96 * c:1024 + 96 * c + 96, :] = o_c[128:224, :]
        out[1792 + 32 * c:1792 + 32 * c + 32, :] = \
            res.results[c]["out_r2T"].T
    return out.reshape(1, S, HID)


# revision 5
# speedup vs baseline: 1.0117x; 1.0117x over previous
"""GQA attention (B=1, S=2048, H=2048, 32 q-heads / 8 kv-heads, hd=64)
on 8 Trainium2 NeuronCores.

Sharding: tensor-parallel over heads for QKV+attention (core c owns
q-heads 4c..4c+3 and kv-head c), then sequence-parallel o_proj via
three AllToAll rounds aligned to q-chunk boundaries (q [0,1024) /
[1024,1536) / [1536,2048)). Per round, core c owns qn={128,64,64}
query rows; each core holds the FULL wo (bf16). Round 0's o_proj
matmuls are interleaved into qchunk 3's tile loop (its A2A completes
mid-attention); rounds 1/2 (M=64) use 2-way PE column tiling + a
transpose-reduce matmul, producing transposed outputs (out_r1T /
out_r2T) that the host fixes up.

Engine queues are FIFO in emission order, so long-dependency DMAs
must not sit ahead of staging DMAs: the rope half-swap and the kv
k-duplication are done as PE permutation matmuls (consts pq/pa/pb)
instead of SBUF-shift DMAs. B-phase activations are grouped (all Ln,
then all Exp) to avoid ACT table thrash against the attention Exp
stream; 1/l uses the fast DVE reciprocal. A tiny AllToAll prewarms
the collective path.
"""
import numpy as np
import sys

sys.path.insert(0, "/opt/trn_rl_repo")

import concourse.bacc as bacc  # noqa: E402
import concourse.mybir as mybir  # noqa: E402
import concourse.tile as tile  # noqa: E402
from concourse import bass_utils  # noqa: E402

f32 = mybir.dt.float32
bf16 = mybir.dt.bfloat16
AF = mybir.ActivationFunctionType
BF16NP = mybir.dt.np(bf16)

N_CORES = 8
S = 2048
HID = 2048
HD = 64
ROPE_THETA = 10000.0
RMS_EPS = 1e-6
SCALING = HD ** -0.5              # 0.125
NK = HID // 128                   # 16 contraction tiles
# rounds: (q_start, per-core qpos)
ROUNDS = ((0, 128), (1024, 64), (1536, 64))

_NC_CACHE = None
LAST_RESULTS = None


def _build():
    nc = bacc.Bacc("TRN2", target_bir_lowering=False, debug=False,
                   num_devices=N_CORES)

    def din(name, shape, dt):
        return nc.dram_tensor(name, shape, dt, kind="ExternalInput").ap()

    xP = din("xP", [2 * NK * 128, 1024], bf16)
    wq0 = din("wq0", [128, HID], bf16)
    wq1 = din("wq1", [128, HID], bf16)
    wkv = din("wkv", [128, HID], bf16)     # [wv | wk] columns pretiled
    wof = din("wof", [128, NK * HID], bf16)  # FULL wo, pretiled
    cos2 = din("cos2", [128, S], bf16)
    ss2 = din("ss2", [128, S], bf16)
    ew_q = din("ew_q", [2, 128], bf16)
    ew_k = din("ew_k", [2, 128], bf16)
    e2 = din("e2", [2, 128], bf16)
    e2t = din("e2t", [128, 2], bf16)
    mask = din("mask", [128, 128], bf16)
    ident = din("ident", [64, 64], bf16)
    r2 = din("r2", [128, 64], bf16)        # R2[64g+j, j] = 1
    pq_ = din("pq_", [128, 128], bf16)     # rope swap perm
    pa_ = din("pa_", [128, 128], bf16)     # kv nrm-dup perm
    pb_ = din("pb_", [128, 128], bf16)     # kv sh-dup perm

    out_rs = nc.dram_tensor("out_rs", [128, S], f32,
                            kind="ExternalOutput").ap()
    out_r1T = nc.dram_tensor("out_r1T", [HID, 64], f32,
                             kind="ExternalOutput").ap()
    out_r2T = nc.dram_tensor("out_r2T", [HID, 64], f32,
                             kind="ExternalOutput").ap()

    with tile.TileContext(nc) as tc:
        with tc.tile_pool(name="consts", bufs=1) as cp, \
             tc.tile_pool(name="dram", bufs=1, space="DRAM") as dp:
            c_wq0 = cp.tile([128, HID], bf16, tag="w")
            c_wq1 = cp.tile([128, HID], bf16, tag="w2")
            c_wkv = cp.tile([128, HID], bf16, tag="w3")
            c_wo = cp.tile([128, NK * HID], bf16, tag="w4")
            c_cos = cp.tile([128, S], bf16, tag="c1")
            c_ss = cp.tile([128, S], bf16, tag="c2")
            c_ewq = cp.tile([2, 128], bf16, tag="c3")
            c_ewk = cp.tile([2, 128], bf16, tag="c4")
            c_e2 = cp.tile([2, 128], bf16, tag="c5")
            c_e2t = cp.tile([128, 2], bf16, tag="c5t")
            c_mask = cp.tile([128, 128], bf16, tag="c6")
            c_id = cp.tile([64, 64], bf16, tag="c7")
            c_r2 = cp.tile([128, 64], bf16, tag="c7r")
            c_pq = cp.tile([128, 128], bf16, tag="cpq")
            c_pa = cp.tile([128, 128], bf16, tag="cpa")
            c_pb = cp.tile([128, 128], bf16, tag="cpb")
            c_eps = cp.tile([2, 1], f32, tag="c8")
            c_scr = cp.tile([128, 640], bf16, tag="c9")

            nc.vector.memset(c_scr[:], 0.0)
            nc.vector.memset(c_eps[:], RMS_EPS)

            pre_in = dp.tile([8, 16], bf16, name="prei", tag="pi")
            pre_out = dp.tile([8, 16], bf16, name="preo", tag="po")
            a2a_in = [dp.tile([S, qn], bf16, name=f"a2ai{r}",
                              tag=f"ai{r}")
                      for r, (_, qn) in enumerate(ROUNDS)]
            a2a_out = [dp.tile([S, qn], bf16, name=f"a2ao{r}",
                               tag=f"ao{r}")
                       for r, (_, qn) in enumerate(ROUNDS)]

            # prewarm the collective path with a tiny AllToAll
            nc.sync.dma_start(pre_in[:], c_scr[0:8, 0:16])
            nc.gpsimd.collective_compute(
                "AllToAll", mybir.AluOpType.bypass,
                replica_groups=[list(range(N_CORES))],
                ins=[pre_in[:, :].opt()], outs=[pre_out[:, :].opt()])

            # weight loads: whole-tensor contiguous, one per queue
            nc.sync.dma_start(c_wq0[:], wq0)
            nc.scalar.dma_start(c_e2t[:], e2t)
            nc.scalar.dma_start(c_ewq[:], ew_q)
            nc.scalar.dma_start(c_ewk[:], ew_k)
            nc.scalar.dma_start(c_id[:], ident)
            nc.scalar.dma_start(c_e2[:], e2)
            nc.scalar.dma_start(c_r2[:], r2)
            nc.scalar.dma_start(c_mask[:], mask)
            nc.scalar.dma_start(c_pq[:], pq_)
            nc.scalar.dma_start(c_pa[:], pa_)
            nc.scalar.dma_start(c_pb[:], pb_)
            nc.scalar.dma_start(c_wq1[:], wq1)
            nc.gpsimd.dma_start(c_wkv[:], wkv)

            qkv = {
                "q0": cp.tile([128, S], bf16, tag="q0", name="q0"),
                "q1": cp.tile([128, S], bf16, tag="q1", name="q1"),
                "kv": cp.tile([128, S], bf16, tag="kv", name="kv"),
            }
            qr0h = [cp.tile([128, 1024], bf16, tag=f"qr0{h}",
                            name=f"qr0{h}") for h in range(2)]
            qr1h = [cp.tile([128, 1024], bf16, tag=f"qr1{h}",
                            name=f"qr1{h}") for h in range(2)]
            krdh = [cp.tile([128, 1024], bf16, tag=f"krd{h}",
                            name=f"krd{h}") for h in range(2)]
            vah = [cp.tile([128, 8 * (HD + 1)], bf16, tag=f"va{h}",
                           name=f"va{h}") for h in range(2)]
            attn_bf = [cp.tile([128, S], bf16, tag=f"abf{i}",
                               name=f"abf{i}") for i in range(2)]

            with tc.tile_pool(name="xt", bufs=4) as xp, \
                 tc.tile_pool(name="sbB", bufs=2) as sbB:

                def phase_a(qh, psA):
                    hs = slice(1024 * qh, 1024 * qh + 1024)
                    pq = [psA.tile([128, 1024], f32, tag="pa",
                                   name=f"pa{qh}_{j}") for j in range(3)]
                    for t in range(NK):
                        xt = xp.tile([128, 1024], bf16, tag="xt")
                        eng = (nc.sync, nc.scalar, nc.gpsimd)[t % 3]
                        xr = (qh * NK + t) * 128
                        eng.dma_start(xt[:], xP[xr:xr + 128, :])
                        st = (t == 0)
                        sp = (t == NK - 1)
                        tc_ = slice(128 * t, 128 * (t + 1))
                        for j, w in ((0, c_wq0), (1, c_wq1), (2, c_wkv)):
                            nc.tensor.matmul(pq[j][:, 0:512], w[:, tc_],
                                             xt[:, 0:512],
                                             start=st, stop=sp)
                            nc.tensor.matmul(pq[j][:, 512:1024],
                                             w[:, tc_], xt[:, 512:1024],
                                             start=st, stop=sp)
                    for j, key in ((0, "q0"), (1, "q1"), (2, "kv")):
                        nc.vector.tensor_copy(qkv[key][:, hs], pq[j][:])

                # --- B-phase, split so ACT ops group by table-set ---
                def b_p1a(qh, si, key, is_kv, psP, ptag):
                    hs = slice(1024 * qh, 1024 * qh + 1024)
                    src = qkv[key]
                    if is_kv:
                        nc.gpsimd.memset(vah[qh][:], 1.0)
                        for lt in range(8):
                            ptr = psP.tile([128, 64], bf16, tag=ptag,
                                           name=f"ptr{qh}_{lt}")
                            nc.tensor.transpose(
                                ptr[:],
                                src[0:64, 1024 * qh + 128 * lt:
                                    1024 * qh + 128 * (lt + 1)],
                                c_id[:])
                            nc.vector.tensor_copy(
                                vah[qh][:, (HD + 1) * lt:
                                        (HD + 1) * lt + HD],
                                ptr[:])
                    sq = sbB.tile([128, 1024], bf16, tag="sq",
                                  bufs=2, name=f"sq{qh}_{si}")
                    nc.vector.tensor_mul(sq[:], src[:, hs], src[:, hs])
                    lnvs = {}
                    for u in range(2):
                        us = slice(512 * u, 512 * u + 512)
                        pss = psP.tile([2, 512], f32, tag=ptag,
                                       name=f"ss{qh}_{si}_{u}")
                        nc.tensor.matmul(pss[:], c_e2t[:], sq[:, us],
                                         start=True, stop=True)
                        lnv = sbB.tile([2, 512], bf16, tag="lnv",
                                       bufs=6, name=f"lnv{qh}{si}{u}")
                        nc.scalar.activation(lnv[:], pss[:], AF.Ln,
                                             scale=1.0 / HD,
                                             bias=c_eps[:])
                        lnvs[u] = lnv
                    return lnvs

                def b_p1b(qh, si, lnvs):
                    rstds = {}
                    for u in range(2):
                        rr = sbB.tile([2, 512], bf16, tag="rstdr",
                                      bufs=6, name=f"rr{qh}{si}{u}")
                        nc.scalar.activation(rr[:], lnvs[u][:],
                                             AF.Exp, scale=-0.5)
                        rstds[u] = rr
                    return rstds

                def b_p2(qh, si, key, ew, dst, is_kv, rstds, psP, ptag):
                    src = qkv[key]
                    for u in range(2):
                        cs = slice(1024 * qh + 512 * u,
                                   1024 * qh + 512 * u + 512)
                        us = slice(512 * u, 512 * u + 512)
                        pb = psP.tile([128, 512], f32, tag=ptag,
                                      name=f"pb{qh}_{si}_{u}")
                        nc.tensor.matmul(pb[:], ew[:], rstds[u][:],
                                         start=True, stop=True)
                        nrm = sbB.tile([128, 512], bf16, tag="nrm",
                                       bufs=4, name=f"nrm{qh}{si}{u}")
                        nc.vector.tensor_mul(nrm[:], src[:, cs], pb[:])
                        if is_kv:
                            pA = psP.tile([128, 512], f32, tag=ptag,
                                          name=f"pA{qh}{si}{u}")
                            nc.tensor.matmul(pA[:], c_pa[:], nrm[:],
                                             start=True, stop=True)
                            pB = psP.tile([128, 512], f32, tag=ptag,
                                          name=f"pB{qh}{si}{u}")
                            nc.tensor.matmul(pB[:], c_pb[:], nrm[:],
                                             start=True, stop=True)
                            srcA, srcB = pA, pB
                        else:
                            pS = psP.tile([128, 512], f32, tag=ptag,
                                          name=f"pS{qh}{si}{u}")
                            nc.tensor.matmul(pS[:], c_pq[:], nrm[:],
                                             start=True, stop=True)
                            srcA, srcB = nrm, pS
                        t2 = sbB.tile([128, 512], f32, tag="t2",
                                      bufs=2, name=f"t2{qh}{si}{u}")
                        nc.vector.tensor_mul(t2[:], srcB[:],
                                             c_ss[:, cs])
                        t1 = sbB.tile([128, 512], f32, tag="sh",
                                      bufs=2, name=f"t1{qh}{si}{u}")
                        nc.vector.tensor_mul(t1[:], srcA[:],
                                             c_cos[:, cs])
                        nc.vector.tensor_add(dst[:, us], t1[:], t2[:])

                B0 = (("kv", c_ewk, True), ("q0", c_ewq, False),
                      ("q1", c_ewq, False))

                def b_group(qh, dsts, psP, ptag):
                    ls = [b_p1a(qh, si, key, ik, psP, ptag)
                          for si, (key, _, ik) in enumerate(B0)]
                    rs = [b_p1b(qh, si, ls[si]) for si in range(3)]
                    for si, (key, ew, ik) in enumerate(B0):
                        b_p2(qh, si, key, ew, dsts[si], ik, rs[si],
                             psP, ptag)

                # ---- scope 1: warmup + A0 + B0-group + A1 ----
                with tc.tile_pool(name="psA", bufs=3,
                                  space="PSUM") as psA, \
                     tc.tile_pool(name="psM", bufs=2,
                                  space="PSUM") as psM:
                    pwm = psM.tile([128, 512], f32, tag="m", name="pwm")
                    for i in range(32):
                        nc.tensor.matmul(pwm[:], c_scr[:, 0:128],
                                         c_scr[:, 128:640],
                                         start=True, stop=True)
                    phase_a(0, psA)
                    nc.gpsimd.dma_start(c_cos[:], cos2)
                    nc.gpsimd.dma_start(c_ss[:], ss2)
                    b_group(0, (krdh[0], qr0h[0], qr1h[0]), psM, "m")
                    phase_a(1, psA)
                    for h in range(8):
                        cs_ = slice(4096 * h, 4096 * (h + 1))
                        nc.gpsimd.dma_start(c_wo[:, cs_], wof[:, cs_])

                # ---- scope 2: B1 + qchunks + A2As + o_proj ----
                with tc.tile_pool(name="sbC", bufs=4) as sbC, \
                     tc.tile_pool(name="atk", bufs=2) as akp, \
                     tc.tile_pool(name="psS", bufs=2,
                                  space="PSUM") as psS, \
                     tc.tile_pool(name="psPV", bufs=2,
                                  space="PSUM") as psPV, \
                     tc.tile_pool(name="psB", bufs=1,
                                  space="PSUM") as psB, \
                     tc.tile_pool(name="psO", bufs=1,
                                  space="PSUM") as psO:

                    def qchunk(q0, W, inter=None):
                        qs = slice(q0, q0 + W)
                        qhh = q0 // 1024
                        qcol0 = q0 - 1024 * qhh
                        ntile = (q0 + W) // 128
                        t0k = q0 // 128
                        rnd = 0 if q0 < 1024 else (1 if q0 < 1536
                                                   else 2)
                        rs_, qn = ROUNDS[rnd]
                        for hp, qrh in ((0, qr0h), (1, qr1h)):
                            qr = qrh[qhh]
                            ppv_a = psPV.tile([65, W], f32, tag="pv",
                                              name=f"pva{q0}_{hp}")
                            ppv_b = psPV.tile([65, W], f32, tag="pv",
                                              name=f"pvb{q0}_{hp}")
                            for t in range(ntile):
                                r = t - t0k
                                off = max(0, r) * 128
                                qlo = qcol0 + off
                                qlen = W - off
                                kh = t // 8
                                krd = krdh[kh]
                                v_aug = vah[kh]
                                tl = t - 8 * kh
                                kc = slice(128 * tl, 128 * (tl + 1))
                                vs = slice((HD + 1) * tl,
                                           (HD + 1) * tl + HD + 1)
                                st = (t == 0)
                                sp = (t == ntile - 1)
                                ps_s = psS.tile([128, 1024], f32,
                                                tag="s")
                                nc.tensor.matmul(
                                    ps_s[:, 0:qlen], krd[0:64, kc],
                                    qr[0:64, qlo:qlo + qlen],
                                    start=True, stop=True)
                                nc.tensor.matmul(
                                    ps_s[:, 512:512 + qlen],
                                    krd[64:128, kc],
                                    qr[64:128, qlo:qlo + qlen],
                                    start=True, stop=True)
                                pt = sbC.tile([128, 1024], bf16,
                                              tag="pt")
                                nc.scalar.activation(
                                    pt[:, 0:512 + qlen],
                                    ps_s[:, 0:512 + qlen],
                                    AF.Exp, scale=SCALING)
                                if r >= 0:
                                    nc.vector.tensor_mul(
                                        pt[:, 0:128], pt[:, 0:128],
                                        c_mask[:])
                                    nc.vector.tensor_mul(
                                        pt[:, 512:640], pt[:, 512:640],
                                        c_mask[:])
                                nc.tensor.matmul(
                                    ppv_a[:, off:W], v_aug[:, vs],
                                    pt[:, 0:qlen], start=st, stop=sp)
                                nc.tensor.matmul(
                                    ppv_b[:, off:W], v_aug[:, vs],
                                    pt[:, 512:512 + qlen],
                                    start=st, stop=sp)
                                if inter is not None and \
                                        (hp, t) >= (0, 2):
                                    inter(5)
                            # normalize + stage
                            nc.vector.tensor_copy(
                                attn_bf[hp][0:64, qs], ppv_a[0:64, :])
                            stgb = sbC.tile([64, W], bf16, tag="stg",
                                            bufs=2,
                                            name=f"sg{q0}_{hp}")
                            nc.vector.tensor_copy(stgb[:],
                                                  ppv_b[0:64, :])
                            nc.sync.dma_start(
                                attn_bf[hp][64:128, qs], stgb[:])
                            la = sbC.tile([65, W], f32, tag="la",
                                          bufs=2, name=f"la{q0}_{hp}")
                            nc.vector.tensor_copy(la[64:65, :],
                                                  ppv_a[64:65, :])
                            lb = sbC.tile([65, W], f32, tag="la",
                                          bufs=2, name=f"lb{q0}_{hp}")
                            nc.vector.tensor_copy(lb[64:65, :],
                                                  ppv_b[64:65, :])
                            lf = sbC.tile([2, W], f32, tag="lf",
                                          bufs=2, name=f"lf{q0}_{hp}")
                            nc.sync.dma_start(lf[0:1, :], la[64:65, :])
                            nc.gpsimd.dma_start(lf[1:2, :],
                                                lb[64:65, :])
                            rlf = sbC.tile([2, W], f32, tag="lf",
                                           bufs=2, name=f"rf{q0}_{hp}")
                            nc.vector.reciprocal_approx_fast(rlf[:],
                                                             lf[:])
                            rlb = sbC.tile([2, W], bf16, tag="rlb",
                                           bufs=2, name=f"rb{q0}_{hp}")
                            nc.vector.tensor_copy(rlb[:], rlf[:])
                            pb = psB.tile([128, W], f32, tag="b",
                                          name=f"qpb{q0}_{hp}")
                            nc.tensor.matmul(pb[:], c_e2[:], rlb[:],
                                             start=True, stop=True)
                            nc.vector.tensor_mul(
                                attn_bf[hp][:, qs],
                                attn_bf[hp][:, qs], pb[:])
                            seng = nc.gpsimd if hp == 0 else nc.sync
                            for c in range(N_CORES):
                                a = max(q0, rs_ + qn * c)
                                b = min(q0 + W, rs_ + qn * (c + 1))
                                if a < b:
                                    rr_ = 256 * c + 128 * hp
                                    seng.dma_start(
                                        a2a_in[rnd][rr_:rr_ + 128,
                                                    a - rs_ - qn * c:
                                                    b - rs_ - qn * c],
                                        attn_bf[hp][:, a:b])

                    def a2a(rnd):
                        nc.gpsimd.collective_compute(
                            "AllToAll",
                            mybir.AluOpType.bypass,
                            replica_groups=[list(range(N_CORES))],
                            ins=[a2a_in[rnd][:, :].opt()],
                            outs=[a2a_out[rnd][:, :].opt()],
                        )

                    def oproj_tail(rnd):
                        outT = out_r1T if rnd == 1 else out_r2T
                        attk = akp.tile([128, NK * 64], bf16,
                                        tag="atk", name=f"atk{rnd}")
                        for kk in range(NK):
                            eng = nc.sync if kk % 2 == 0 else nc.scalar
                            eng.dma_start(
                                attk[:, 64 * kk:64 * (kk + 1)],
                                a2a_out[rnd][128 * kk:
                                             128 * (kk + 1), :])
                        pos = psO.tile([128, 512], f32, tag="o",
                                       name=f"pos{rnd}")
                        for sub in range(8):
                            h = 256 * (sub % 2)
                            wcs = 256 * sub
                            for g in range(2):
                                for i in range(8):
                                    kk = 8 * g + i
                                    nc.tensor.matmul(
                                        pos[64 * g:64 * g + 64,
                                            h:h + 256],
                                        attk[:, 64 * kk:64 * kk + 64],
                                        c_wo[:, HID * kk + wcs:
                                             HID * kk + wcs + 256],
                                        start=(i == 0), stop=(i == 7),
                                        tile_position=(0, 64 * g))
                            ost4 = sbC.tile([128, 256], bf16,
                                            tag="ost", bufs=3,
                                            name=f"o4_{rnd}_{sub}")
                            nc.vector.tensor_copy(ost4[:],
                                                  pos[:, h:h + 256])
                            pr = psB.tile([128, 128], f32, tag="b",
                                          name=f"pr{rnd}_{sub}")
                            nc.tensor.matmul(pr[:, 0:64],
                                             ost4[:, 0:128], c_r2[:],
                                             start=True, stop=True)
                            nc.tensor.matmul(pr[:, 64:128],
                                             ost4[:, 128:256], c_r2[:],
                                             start=True, stop=True)
                            o2 = sbC.tile([128, 128], f32, tag="ost",
                                          bufs=3,
                                          name=f"o2_{rnd}_{sub}")
                            nc.vector.tensor_copy(o2[:], pr[:])
                            e1 = nc.sync if sub % 2 == 0 else nc.scalar
                            e2_ = nc.scalar if sub % 2 == 0 else \
                                nc.sync
                            e1.dma_start(outT[wcs:wcs + 128, :],
                                         o2[:, 0:64])
                            e2_.dma_start(outT[wcs + 128:wcs + 256, :],
                                          o2[:, 64:128])

                    qchunk(0, 512)
                    b_group(1, (krdh[1], qr0h[1], qr1h[1]), psS, "s")
                    qchunk(512, 512)
                    a2a(0)
                    # prefetch round-0 attk on the (idle) scalar queue
                    attk0 = akp.tile([128, NK * 128], bf16, tag="atk",
                                     name="atk0")
                    for kk in range(NK):
                        nc.scalar.dma_start(
                            attk0[:, 128 * kk:128 * (kk + 1)],
                            a2a_out[0][128 * kk:128 * (kk + 1), :])
                    pos0 = psO.tile([128, 512], f32, tag="o",
                                    name="pos0")

                    # round-0 o_proj emission steps, interleaved
                    # into qchunk(1536)'s tile loop
                    def op0_steps():
                        for sub in range(8):
                            h = 256 * (sub % 2)
                            wcs = 256 * sub
                            for kk in range(NK):
                                yield ("mm", sub, h, wcs, kk)
                            yield ("fin", sub, h, wcs, 0)
                    _op0 = op0_steps()
                    _done = [False]

                    def op0_drain(n):
                        for _ in range(n):
                            step = next(_op0, None)
                            if step is None:
                                _done[0] = True
                                return
                            kind, sub, h, wcs, kk = step
                            if kind == "mm":
                                nc.tensor.matmul(
                                    pos0[:, h:h + 256],
                                    attk0[:, 128 * kk:128 * (kk + 1)],
                                    c_wo[:, HID * kk + wcs:
                                         HID * kk + wcs + 256],
                                    start=(kk == 0),
                                    stop=(kk == NK - 1))
                            else:
                                ost = sbC.tile([128, 256], f32,
                                               tag="ost", bufs=3,
                                               name=f"ost0_{sub}")
                                nc.vector.tensor_copy(
                                    ost[:], pos0[:, h:h + 256])
                                eng = nc.scalar if sub % 2 == 0 else \
                                    nc.sync
                                eng.dma_start(
                                    out_rs[:, wcs:wcs + 256], ost[:])

                    qchunk(1024, 512)
                    a2a(1)
                    qchunk(1536, 512, inter=op0_drain)
                    while not _done[0]:
                        op0_drain(8)
                    a2a(2)
                    oproj_tail(1)
                    oproj_tail(2)

    nc.compile()
    return nc


def _host_prep(hidden_states, position_ids, wq, wk, wv, wo, q_ln_w, k_ln_w):
    x = np.asarray(hidden_states, dtype=np.float32)[0]        # [S, HID]
    xT = np.ascontiguousarray(x.T).astype(BF16NP)             # [HID, S]
    xP = np.ascontiguousarray(
        xT.reshape(NK, 128, 2, 1024).transpose(2, 0, 1, 3)
    ).reshape(2 * NK * 128, 1024)
    pos = np.asarray(position_ids)[0].astype(np.float32)      # [S]
    inv = 1.0 / (ROPE_THETA ** (np.arange(0, HD, 2, dtype=np.float32) / HD))
    ang = pos[:, None] * inv[None, :]                         # [S, 32]
    emb = np.concatenate([ang, ang], axis=1)                  # [S, 64]
    cosT = np.cos(emb).T.astype(np.float32)                   # [64, S]
    sinT = np.sin(emb).T.astype(np.float32)
    ss = sinT.copy()
    ss[0:32] = -sinT[0:32]
    cos2 = np.tile(cosT, (2, 1))
    ss2 = np.tile(ss, (2, 1))

    e2 = np.zeros((2, 128), dtype=np.float32)
    e2[0, 0:64] = 1.0
    e2[1, 64:128] = 1.0
    ew_q = np.zeros((2, 128), dtype=np.float32)
    ew_q[0, 0:64] = q_ln_w
    ew_q[1, 64:128] = q_ln_w
    ew_k = np.zeros((2, 128), dtype=np.float32)
    ew_k[1, 64:128] = k_ln_w
    msk = (np.arange(128)[:, None] <= np.arange(128)[None, :]) \
        .astype(np.float32)
    ident = np.eye(64, dtype=np.float32)
    r2 = np.zeros((128, 64), dtype=np.float32)
    for g in range(2):
        r2[64 * g + np.arange(64), np.arange(64)] = 1.0
    m_ = np.arange(128)
    pq_ = np.zeros((128, 128), dtype=np.float32)
    pq_[m_ ^ 32, m_] = 1.0
    pa_ = np.zeros((128, 128), dtype=np.float32)
    pa_[64 + (m_ & 63), m_] = 1.0
    pb_ = np.zeros((128, 128), dtype=np.float32)
    pb_[64 + ((m_ & 63) ^ 32), m_] = 1.0

    wq_ = np.asarray(wq, dtype=np.float32)
    wk_ = np.asarray(wk, dtype=np.float32)
    wv_ = np.asarray(wv, dtype=np.float32)
    wo_ = np.asarray(wo, dtype=np.float32)

    def pretile(w):  # [HID, N] -> [128, NK*N] ktile-blocked
        n = w.shape[1]
        return np.ascontiguousarray(
            w.reshape(NK, 128, n).transpose(1, 0, 2).reshape(128, NK * n))

    wof = pretile(wo_).astype(BF16NP)

    in_maps = []
    for c in range(N_CORES):
        qcols = slice(256 * c, 256 * (c + 1))
        kvcols = slice(64 * c, 64 * (c + 1))
        wq_c = np.ascontiguousarray(wq_[:, qcols])
        wkv_c = np.concatenate([wv_[:, kvcols], wk_[:, kvcols]], axis=1)
        in_maps.append({
            "xP": xP,
            "wq0": pretile(wq_c[:, 0:128]).astype(BF16NP),
            "wq1": pretile(wq_c[:, 128:256]).astype(BF16NP),
            "wkv": pretile(wkv_c).astype(BF16NP),
            "wof": wof,
            "cos2": cos2.astype(BF16NP),
            "ss2": ss2.astype(BF16NP),
            "ew_q": ew_q.astype(BF16NP),
            "ew_k": ew_k.astype(BF16NP),
            "e2": e2.astype(BF16NP),
            "e2t": np.ascontiguousarray(e2.T).astype(BF16NP),
            "mask": msk.astype(BF16NP),
            "ident": ident.astype(BF16NP),
            "r2": r2.astype(BF16NP),
            "pq_": pq_.astype(BF16NP),
            "pa_": pa_.astype(BF16NP),
            "pb_": pb_.astype(BF16NP),
        })
    return in_maps


def kernel(hidden_states, position_ids, wq, wk, wv, wo, q_ln_w, k_ln_w):
    global _NC_CACHE, LAST_RESULTS
    if _NC_CACHE is None:
        _NC_CACHE = _build()
    nc = _NC_CACHE
    in_maps = _host_prep(hidden_states, position_ids, wq, wk, wv, wo,
                         q_ln_w, k_ln_w)
    res = bass_utils.run_bass_kernel_spmd(
        nc, in_maps, core_ids=list(range(N_CORES)))
    LAST_RESULTS = res
    out = np.empty((S, HID), dtype=np.float32)
    for c in range(N_CORES):
        out[128 * c:128 * c + 128, :] = res.results[c]["out_rs"]
        out[1024 + 64 * c:1024 + 64 * c + 64, :] = \
            res.results[c]["out_r1T"].T
        out[1536 + 64 * c:1536 + 64 * c + 64, :] = \
            res.results[c]["out_r2T"].T
    return out.reshape(1, S, HID)


# revision 11
# speedup vs baseline: 1.0345x; 1.0225x over previous
"""GQA attention (B=1, S=2048, H=2048, 32 q-heads / 8 kv-heads, hd=64)
on 8 Trainium2 NeuronCores.

Sharding: tensor-parallel over heads for QKV+attention (core c owns
q-heads 4c..4c+3 and kv-head c), then sequence-parallel o_proj via
three AllToAll rounds aligned to q-chunk boundaries (q [0,1024) /
[1024,1536) / [1536,2048)). Per round, core c owns qn={128,64,64}
query rows; each core holds the FULL wo (bf16). Round 0's o_proj
matmuls are interleaved into qchunk 3's tile loop (its A2A completes
mid-attention); rounds 1/2 (M=64) use 2-way PE column tiling + a
transpose-reduce matmul, producing transposed outputs (out_r1T /
out_r2T) that the host fixes up.

Engine queues are FIFO in emission order, so long-dependency DMAs
must not sit ahead of staging DMAs: the rope half-swap and the kv
k-duplication are done as PE permutation matmuls (consts pq/pa/pb)
instead of SBUF-shift DMAs. B-phase activations are grouped (all Ln,
then all Exp) to avoid ACT table thrash against the attention Exp
stream; 1/l uses the fast DVE reciprocal. A tiny AllToAll prewarms
the collective path.
"""
import numpy as np
import sys

sys.path.insert(0, "/opt/trn_rl_repo")

import concourse.bacc as bacc  # noqa: E402
import concourse.mybir as mybir  # noqa: E402
import concourse.tile as tile  # noqa: E402
from concourse import bass_utils  # noqa: E402

f32 = mybir.dt.float32
bf16 = mybir.dt.bfloat16
AF = mybir.ActivationFunctionType
BF16NP = mybir.dt.np(bf16)

N_CORES = 8
S = 2048
HID = 2048
HD = 64
ROPE_THETA = 10000.0
RMS_EPS = 1e-6
SCALING = HD ** -0.5              # 0.125
NK = HID // 128                   # 16 contraction tiles
# rounds: (q_start, per-core qpos)
ROUNDS = ((0, 128), (1024, 64), (1536, 64))

_NC_CACHE = None
LAST_RESULTS = None


def _build():
    nc = bacc.Bacc("TRN2", target_bir_lowering=False, debug=False,
                   num_devices=N_CORES)

    def din(name, shape, dt):
        return nc.dram_tensor(name, shape, dt, kind="ExternalInput").ap()

    xP = din("xP", [2 * NK * 128, 1024], bf16)
    wq0 = din("wq0", [128, HID], bf16)
    wq1 = din("wq1", [128, HID], bf16)
    wkv = din("wkv", [128, HID], bf16)     # [wv | wk] columns pretiled
    wof = din("wof", [128, NK * HID], bf16)  # FULL wo, pretiled
    cos2 = din("cos2", [128, S], bf16)
    ss2 = din("ss2", [128, S], bf16)
    ew_q = din("ew_q", [2, 128], bf16)
    ew_k = din("ew_k", [2, 128], bf16)
    e2 = din("e2", [2, 128], bf16)
    e2t = din("e2t", [128, 2], bf16)
    mask = din("mask", [128, 128], bf16)
    ident = din("ident", [64, 64], bf16)
    r2 = din("r2", [128, 64], bf16)        # R2[64g+j, j] = 1
    pq_ = din("pq_", [128, 128], bf16)     # rope swap perm
    pa_ = din("pa_", [128, 128], bf16)     # kv nrm-dup perm
    pb_ = din("pb_", [128, 128], bf16)     # kv sh-dup perm

    out_rs = nc.dram_tensor("out_rs", [128, S], f32,
                            kind="ExternalOutput").ap()
    out_r1T = nc.dram_tensor("out_r1T", [HID, 64], f32,
                             kind="ExternalOutput").ap()
    out_r2T = nc.dram_tensor("out_r2T", [HID, 64], f32,
                             kind="ExternalOutput").ap()

    with tile.TileContext(nc) as tc:
        with tc.tile_pool(name="consts", bufs=1) as cp, \
             tc.tile_pool(name="dram", bufs=1, space="DRAM") as dp:
            c_wq0 = cp.tile([128, HID], bf16, tag="w")
            c_wq1 = cp.tile([128, HID], bf16, tag="w2")
            c_wkv = cp.tile([128, HID], bf16, tag="w3")
            c_wo = cp.tile([128, NK * HID], bf16, tag="w4")
            c_cos = cp.tile([128, S], bf16, tag="c1")
            c_ss = cp.tile([128, S], bf16, tag="c2")
            c_ewq = cp.tile([2, 128], bf16, tag="c3")
            c_ewk = cp.tile([2, 128], bf16, tag="c4")
            c_e2 = cp.tile([2, 128], bf16, tag="c5")
            c_e2t = cp.tile([128, 2], bf16, tag="c5t")
            c_mask = cp.tile([128, 128], bf16, tag="c6")
            c_id = cp.tile([64, 64], bf16, tag="c7")
            c_r2 = cp.tile([128, 64], bf16, tag="c7r")
            c_pq = cp.tile([128, 128], bf16, tag="cpq")
            c_pa = cp.tile([128, 128], bf16, tag="cpa")
            c_pb = cp.tile([128, 128], bf16, tag="cpb")
            c_eps = cp.tile([2, 1], f32, tag="c8")
            c_scr = cp.tile([128, 640], bf16, tag="c9")

            nc.vector.memset(c_scr[:], 0.0)
            nc.vector.memset(c_eps[:], RMS_EPS)

            pre_in = dp.tile([8, 16], bf16, name="prei", tag="pi")
            pre_out = dp.tile([8, 16], bf16, name="preo", tag="po")
            a2a_in = [dp.tile([S, qn], bf16, name=f"a2ai{r}",
                              tag=f"ai{r}")
                      for r, (_, qn) in enumerate(ROUNDS)]
            a2a_out = [dp.tile([S, qn], bf16, name=f"a2ao{r}",
                               tag=f"ao{r}")
                       for r, (_, qn) in enumerate(ROUNDS)]

            # prewarm the collective path with a tiny AllToAll
            nc.sync.dma_start(pre_in[:], c_scr[0:8, 0:16])
            nc.gpsimd.collective_compute(
                "AllToAll", mybir.AluOpType.bypass,
                replica_groups=[list(range(N_CORES))],
                ins=[pre_in[:, :].opt()], outs=[pre_out[:, :].opt()])

            # weight loads first (halves, so ktile 0-7 weights land
            # early); small consts after the first halves
            nc.sync.dma_start(c_wq0[:, 0:1024], wq0[:, 0:1024])
            nc.scalar.dma_start(c_wq1[:, 0:1024], wq1[:, 0:1024])
            nc.gpsimd.dma_start(c_wkv[:, 0:1024], wkv[:, 0:1024])
            nc.sync.dma_start(c_wq0[:, 1024:2048], wq0[:, 1024:2048])
            nc.scalar.dma_start(c_wq1[:, 1024:2048], wq1[:, 1024:2048])
            nc.gpsimd.dma_start(c_wkv[:, 1024:2048], wkv[:, 1024:2048])
            nc.scalar.dma_start(c_e2t[:], e2t)
            nc.scalar.dma_start(c_ewq[:], ew_q)
            nc.scalar.dma_start(c_ewk[:], ew_k)
            nc.scalar.dma_start(c_id[:], ident)
            nc.scalar.dma_start(c_e2[:], e2)
            nc.scalar.dma_start(c_r2[:], r2)
            nc.scalar.dma_start(c_mask[:], mask)
            nc.scalar.dma_start(c_pq[:], pq_)
            nc.scalar.dma_start(c_pa[:], pa_)
            nc.scalar.dma_start(c_pb[:], pb_)

            qkv = {
                "q0": cp.tile([128, S], bf16, tag="q0", name="q0"),
                "q1": cp.tile([128, S], bf16, tag="q1", name="q1"),
                "kv": cp.tile([128, S], bf16, tag="kv", name="kv"),
            }
            qr0h = [cp.tile([128, 1024], bf16, tag=f"qr0{h}",
                            name=f"qr0{h}") for h in range(2)]
            qr1h = [cp.tile([128, 1024], bf16, tag=f"qr1{h}",
                            name=f"qr1{h}") for h in range(2)]
            krdh = [cp.tile([128, 1024], bf16, tag=f"krd{h}",
                            name=f"krd{h}") for h in range(2)]
            vah = [cp.tile([128, 8 * (HD + 1)], bf16, tag=f"va{h}",
                           name=f"va{h}") for h in range(2)]
            attn_bf = [cp.tile([128, S], bf16, tag=f"abf{i}",
                               name=f"abf{i}") for i in range(2)]

            with tc.tile_pool(name="xt", bufs=4) as xp, \
                 tc.tile_pool(name="sbB", bufs=2) as sbB:

                def phase_a(qh, psA):
                    hs = slice(1024 * qh, 1024 * qh + 1024)
                    pq = [psA.tile([128, 1024], f32, tag="pa",
                                   name=f"pa{qh}_{j}") for j in range(3)]
                    for t in range(NK):
                        xt = xp.tile([128, 1024], bf16, tag="xt")
                        eng = (nc.sync, nc.scalar, nc.gpsimd)[t % 3]
                        xr = (qh * NK + t) * 128
                        eng.dma_start(xt[:], xP[xr:xr + 128, :])
                        st = (t == 0)
                        sp = (t == NK - 1)
                        tc_ = slice(128 * t, 128 * (t + 1))
                        for j, w in ((0, c_wq0), (1, c_wq1), (2, c_wkv)):
                            nc.tensor.matmul(pq[j][:, 0:512], w[:, tc_],
                                             xt[:, 0:512],
                                             start=st, stop=sp)
                            nc.tensor.matmul(pq[j][:, 512:1024],
                                             w[:, tc_], xt[:, 512:1024],
                                             start=st, stop=sp)
                    for j, key in ((0, "q0"), (1, "q1"), (2, "kv")):
                        nc.vector.tensor_copy(qkv[key][:, hs], pq[j][:])

                # --- B-phase, split so ACT ops group by table-set ---
                def b_p1a(qh, si, key, is_kv, psP, ptag):
                    hs = slice(1024 * qh, 1024 * qh + 1024)
                    src = qkv[key]
                    if is_kv:
                        nc.gpsimd.memset(vah[qh][:], 1.0)
                        for lt in range(8):
                            ptr = psP.tile([128, 64], bf16, tag=ptag,
                                           name=f"ptr{qh}_{lt}")
                            nc.tensor.transpose(
                                ptr[:],
                                src[0:64, 1024 * qh + 128 * lt:
                                    1024 * qh + 128 * (lt + 1)],
                                c_id[:])
                            nc.vector.tensor_copy(
                                vah[qh][:, (HD + 1) * lt:
                                        (HD + 1) * lt + HD],
                                ptr[:])
                    sq = sbB.tile([128, 1024], bf16, tag="sq",
                                  bufs=2, name=f"sq{qh}_{si}")
                    nc.vector.tensor_mul(sq[:], src[:, hs], src[:, hs])
                    lnvs = {}
                    for u in range(2):
                        us = slice(512 * u, 512 * u + 512)
                        pss = psP.tile([2, 512], f32, tag=ptag,
                                       name=f"ss{qh}_{si}_{u}")
                        nc.tensor.matmul(pss[:], c_e2t[:], sq[:, us],
                                         start=True, stop=True)
                        lnv = sbB.tile([2, 512], bf16, tag="lnv",
                                       bufs=6, name=f"lnv{qh}{si}{u}")
                        nc.scalar.activation(lnv[:], pss[:], AF.Ln,
                                             scale=1.0 / HD,
                                             bias=c_eps[:])
                        lnvs[u] = lnv
                    return lnvs

                def b_p1b(qh, si, lnvs):
                    rstds = {}
                    for u in range(2):
                        rr = sbB.tile([2, 512], bf16, tag="rstdr",
                                      bufs=6, name=f"rr{qh}{si}{u}")
                        nc.scalar.activation(rr[:], lnvs[u][:],
                                             AF.Exp, scale=-0.5)
                        rstds[u] = rr
                    return rstds

                def b_p2(qh, si, key, ew, dst, is_kv, rstds, psP, ptag):
                    src = qkv[key]
                    for u in range(2):
                        cs = slice(1024 * qh + 512 * u,
                                   1024 * qh + 512 * u + 512)
                        us = slice(512 * u, 512 * u + 512)
                        pb = psP.tile([128, 512], f32, tag=ptag,
                                      name=f"pb{qh}_{si}_{u}")
                        nc.tensor.matmul(pb[:], ew[:], rstds[u][:],
                                         start=True, stop=True)
                        nrm = sbB.tile([128, 512], bf16, tag="nrm",
                                       bufs=4, name=f"nrm{qh}{si}{u}")
                        nc.vector.tensor_mul(nrm[:], src[:, cs], pb[:])
                        if is_kv:
                            pA = psP.tile([128, 512], f32, tag=ptag,
                                          name=f"pA{qh}{si}{u}")
                            nc.tensor.matmul(pA[:], c_pa[:], nrm[:],
                                             start=True, stop=True)
                            pB = psP.tile([128, 512], f32, tag=ptag,
                                          name=f"pB{qh}{si}{u}")
                            nc.tensor.matmul(pB[:], c_pb[:], nrm[:],
                                             start=True, stop=True)
                            srcA, srcB = pA, pB
                        else:
                            pS = psP.tile([128, 512], f32, tag=ptag,
                                          name=f"pS{qh}{si}{u}")
                            nc.tensor.matmul(pS[:], c_pq[:], nrm[:],
                                             start=True, stop=True)
                            srcA, srcB = nrm, pS
                        t2 = sbB.tile([128, 512], f32, tag="t2",
                                      bufs=2, name=f"t2{qh}{si}{u}")
                        nc.vector.tensor_mul(t2[:], srcB[:],
                                             c_ss[:, cs])
                        t1 = sbB.tile([128, 512], f32, tag="sh",
                                      bufs=2, name=f"t1{qh}{si}{u}")
                        nc.vector.tensor_mul(t1[:], srcA[:],
                                             c_cos[:, cs])
                        nc.vector.tensor_add(dst[:, us], t1[:], t2[:])

                B0 = (("kv", c_ewk, True), ("q0", c_ewq, False),
                      ("q1", c_ewq, False))

                def b_group(qh, dsts, psP, ptag):
                    ls = [b_p1a(qh, si, key, ik, psP, ptag)
                          for si, (key, _, ik) in enumerate(B0)]
                    rs = [b_p1b(qh, si, ls[si]) for si in range(3)]
                    for si, (key, ew, ik) in enumerate(B0):
                        b_p2(qh, si, key, ew, dsts[si], ik, rs[si],
                             psP, ptag)

                # ---- scope 1: warmup + A0 + B0-group + A1 ----
                with tc.tile_pool(name="psA", bufs=3,
                                  space="PSUM") as psA, \
                     tc.tile_pool(name="psM", bufs=2,
                                  space="PSUM") as psM:
                    pwm = psM.tile([128, 512], f32, tag="m", name="pwm")
                    for i in range(32):
                        nc.tensor.matmul(pwm[:], c_scr[:, 0:128],
                                         c_scr[:, 128:640],
                                         start=True, stop=True)
                    phase_a(0, psA)
                    nc.gpsimd.dma_start(c_cos[:], cos2)
                    nc.gpsimd.dma_start(c_ss[:], ss2)
                    b_group(0, (krdh[0], qr0h[0], qr1h[0]), psM, "m")
                    phase_a(1, psA)
                    for h in range(8):
                        cs_ = slice(4096 * h, 4096 * (h + 1))
                        nc.gpsimd.dma_start(c_wo[:, cs_], wof[:, cs_])

                # ---- scope 2: B1 + qchunks + A2As + o_proj ----
                with tc.tile_pool(name="sbC", bufs=4) as sbC, \
                     tc.tile_pool(name="atk", bufs=2) as akp, \
                     tc.tile_pool(name="psS", bufs=2,
                                  space="PSUM") as psS, \
                     tc.tile_pool(name="psPV", bufs=2,
                                  space="PSUM") as psPV, \
                     tc.tile_pool(name="psB", bufs=1,
                                  space="PSUM") as psB, \
                     tc.tile_pool(name="psO", bufs=1,
                                  space="PSUM") as psO:

                    def qchunk(q0, W, inter=None):
                        qs = slice(q0, q0 + W)
                        qhh = q0 // 1024
                        qcol0 = q0 - 1024 * qhh
                        ntile = (q0 + W) // 128
                        t0k = q0 // 128
                        rnd = 0 if q0 < 1024 else (1 if q0 < 1536
                                                   else 2)
                        rs_, qn = ROUNDS[rnd]
                        for hp, qrh in ((0, qr0h), (1, qr1h)):
                            qr = qrh[qhh]
                            ppv_a = psPV.tile([65, W], f32, tag="pv",
                                              name=f"pva{q0}_{hp}")
                            ppv_b = psPV.tile([65, W], f32, tag="pv",
                                              name=f"pvb{q0}_{hp}")

                            def emit_pv(p):
                                pt, off, qlen, vs, va, st, sp = p
                                nc.tensor.matmul(
                                    ppv_a[:, off:W], va[:, vs],
                                    pt[:, 0:qlen], start=st, stop=sp)
                                nc.tensor.matmul(
                                    ppv_b[:, off:W], va[:, vs],
                                    pt[:, 512:512 + qlen],
                                    start=st, stop=sp)

                            pvq = []
                            for t in range(ntile):
                                r = t - t0k
                                off = max(0, r) * 128
                                qlo = qcol0 + off
                                qlen = W - off
                                kh = t // 8
                                krd = krdh[kh]
                                v_aug = vah[kh]
                                tl = t - 8 * kh
                                kc = slice(128 * tl, 128 * (tl + 1))
                                vs = slice((HD + 1) * tl,
                                           (HD + 1) * tl + HD + 1)
                                st = (t == 0)
                                sp = (t == ntile - 1)
                                ps_s = psS.tile([128, 1024], f32,
                                                tag="s")
                                nc.tensor.matmul(
                                    ps_s[:, 0:qlen], krd[0:64, kc],
                                    qr[0:64, qlo:qlo + qlen],
                                    start=True, stop=True)
                                nc.tensor.matmul(
                                    ps_s[:, 512:512 + qlen],
                                    krd[64:128, kc],
                                    qr[64:128, qlo:qlo + qlen],
                                    start=True, stop=True)
                                pt = sbC.tile([128, 1024], bf16,
                                              tag="pt")
                                nc.scalar.activation(
                                    pt[:, 0:512 + qlen],
                                    ps_s[:, 0:512 + qlen],
                                    AF.Exp, scale=SCALING)
                                if r >= 0:
                                    nc.vector.tensor_mul(
                                        pt[:, 0:128], pt[:, 0:128],
                                        c_mask[:])
                                    nc.vector.tensor_mul(
                                        pt[:, 512:640], pt[:, 512:640],
                                        c_mask[:])
                                pvq.append((pt, off, qlen, vs,
                                            v_aug, st, sp))
                                if len(pvq) > 1:
                                    emit_pv(pvq.pop(0))
                                if inter is not None and \
                                        (hp, t) >= (0, 10):
                                    inter(7)
                            emit_pv(pvq.pop(0))
                            # normalize + stage
                            nc.vector.tensor_copy(
                                attn_bf[hp][0:64, qs], ppv_a[0:64, :])
                            stgb = sbC.tile([64, W], bf16, tag="stg",
                                            bufs=2,
                                            name=f"sg{q0}_{hp}")
                            nc.vector.tensor_copy(stgb[:],
                                                  ppv_b[0:64, :])
                            nc.sync.dma_start(
                                attn_bf[hp][64:128, qs], stgb[:])
                            la = sbC.tile([65, W], f32, tag="la",
                                          bufs=2, name=f"la{q0}_{hp}")
                            nc.vector.tensor_copy(la[64:65, :],
                                                  ppv_a[64:65, :])
                            lb = sbC.tile([65, W], f32, tag="la",
                                          bufs=2, name=f"lb{q0}_{hp}")
                            nc.vector.tensor_copy(lb[64:65, :],
                                                  ppv_b[64:65, :])
                            lf = sbC.tile([2, W], f32, tag="lf",
                                          bufs=2, name=f"lf{q0}_{hp}")
                            nc.sync.dma_start(lf[0:1, :], la[64:65, :])
                            nc.gpsimd.dma_start(lf[1:2, :],
                                                lb[64:65, :])
                            rlf = sbC.tile([2, W], f32, tag="lf",
                                           bufs=2, name=f"rf{q0}_{hp}")
                            nc.vector.reciprocal_approx_fast(rlf[:],
                                                             lf[:])
                            rlb = sbC.tile([2, W], bf16, tag="rlb",
                                           bufs=2, name=f"rb{q0}_{hp}")
                            nc.vector.tensor_copy(rlb[:], rlf[:])
                            pb = psB.tile([128, W], f32, tag="b",
                                          name=f"qpb{q0}_{hp}")
                            nc.tensor.matmul(pb[:], c_e2[:], rlb[:],
                                             start=True, stop=True)
                            nc.vector.tensor_mul(
                                attn_bf[hp][:, qs],
                                attn_bf[hp][:, qs], pb[:])
                            seng = nc.gpsimd if hp == 0 else nc.sync
                            for c in range(N_CORES):
                                a = max(q0, rs_ + qn * c)
                                b = min(q0 + W, rs_ + qn * (c + 1))
                                if a < b:
                                    rr_ = 256 * c + 128 * hp
                                    seng.dma_start(
                                        a2a_in[rnd][rr_:rr_ + 128,
                                                    a - rs_ - qn * c:
                                                    b - rs_ - qn * c],
                                        attn_bf[hp][:, a:b])

                    def a2a(rnd):
                        nc.gpsimd.collective_compute(
                            "AllToAll",
                            mybir.AluOpType.bypass,
                            replica_groups=[list(range(N_CORES))],
                            ins=[a2a_in[rnd][:, :].opt()],
                            outs=[a2a_out[rnd][:, :].opt()],
                        )

                    def oproj_tail(rnd):
                        outT = out_r1T if rnd == 1 else out_r2T
                        attk = akp.tile([128, NK * 64], bf16,
                                        tag="atk", name=f"atk{rnd}")
                        for kk in range(NK):
                            eng = nc.sync if kk % 2 == 0 else nc.scalar
                            eng.dma_start(
                                attk[:, 64 * kk:64 * (kk + 1)],
                                a2a_out[rnd][128 * kk:
                                             128 * (kk + 1), :])
                        pos = psO.tile([128, 512], f32, tag="o",
                                       name=f"pos{rnd}")

                        def flush(p):
                            sub, wcs, ost4 = p
                            pr = psB.tile([128, 128], f32, tag="b",
                                          name=f"pr{rnd}_{sub}")
                            nc.tensor.matmul(pr[:, 0:64],
                                             ost4[:, 0:128], c_r2[:],
                                             start=True, stop=True)
                            nc.tensor.matmul(pr[:, 64:128],
                                             ost4[:, 128:256],
                                             c_r2[:],
                                             start=True, stop=True)
                            o2 = sbC.tile([128, 128], f32, tag="ost",
                                          bufs=4,
                                          name=f"o2_{rnd}_{sub}")
                            nc.vector.tensor_copy(o2[:], pr[:])
                            e1 = nc.sync if sub % 2 == 0 else nc.scalar
                            e2_ = nc.scalar if sub % 2 == 0 else \
                                nc.sync
                            e1.dma_start(outT[wcs:wcs + 128, :],
                                         o2[:, 0:64])
                            e2_.dma_start(outT[wcs + 128:wcs + 256, :],
                                          o2[:, 64:128])

                        pend = None
                        for sub in range(8):
                            h = 256 * (sub % 2)
                            wcs = 256 * sub
                            for g in range(2):
                                for i in range(8):
                                    kk = 8 * g + i
                                    nc.tensor.matmul(
                                        pos[64 * g:64 * g + 64,
                                            h:h + 256],
                                        attk[:, 64 * kk:64 * kk + 64],
                                        c_wo[:, HID * kk + wcs:
                                             HID * kk + wcs + 256],
                                        start=(i == 0), stop=(i == 7),
                                        tile_position=(0, 64 * g))
                            ost4 = sbC.tile([128, 256], bf16,
                                            tag="ost", bufs=4,
                                            name=f"o4_{rnd}_{sub}")
                            nc.vector.tensor_copy(ost4[:],
                                                  pos[:, h:h + 256])
                            if pend is not None:
                                flush(pend)
                            pend = (sub, wcs, ost4)
                        flush(pend)

                    qchunk(0, 512)
                    b_group(1, (krdh[1], qr0h[1], qr1h[1]), psS, "s")
                    qchunk(512, 512)
                    a2a(0)
                    # prefetch round-0 attk on the (idle) scalar queue
                    attk0 = akp.tile([128, NK * 128], bf16, tag="atk",
                                     name="atk0")
                    for kk in range(NK):
                        nc.scalar.dma_start(
                            attk0[:, 128 * kk:128 * (kk + 1)],
                            a2a_out[0][128 * kk:128 * (kk + 1), :])
                    pos0 = psO.tile([128, 512], f32, tag="o",
                                    name="pos0")

                    # round-0 o_proj emission steps, interleaved
                    # into qchunk(1536)'s tile loop
                    def op0_steps():
                        for sub in range(8):
                            h = 256 * (sub % 2)
                            wcs = 256 * sub
                            for kk in range(NK):
                                yield ("mm", sub, h, wcs, kk)
                            yield ("fin", sub, h, wcs, 0)
                    _op0 = op0_steps()
                    _done = [False]

                    def op0_drain(n):
                        for _ in range(n):
                            step = next(_op0, None)
                            if step is None:
                                _done[0] = True
                                return
                            kind, sub, h, wcs, kk = step
                            if kind == "mm":
                                nc.tensor.matmul(
                                    pos0[:, h:h + 256],
                                    attk0[:, 128 * kk:128 * (kk + 1)],
                                    c_wo[:, HID * kk + wcs:
                                         HID * kk + wcs + 256],
                                    start=(kk == 0),
                                    stop=(kk == NK - 1))
                            else:
                                ost = sbC.tile([128, 256], f32,
                                               tag="ost", bufs=4,
                                               name=f"ost0_{sub}")
                                nc.vector.tensor_copy(
                                    ost[:], pos0[:, h:h + 256])
                                nc.scalar.dma_start(
                                    out_rs[:, wcs:wcs + 256], ost[:])

                    qchunk(1024, 512)
                    a2a(1)
                    qchunk(1536, 512, inter=op0_drain)
                    while not _done[0]:
                        op0_drain(8)
                    a2a(2)
                    oproj_tail(1)
                    oproj_tail(2)

    nc.compile()
    return nc


def _host_prep(hidden_states, position_ids, wq, wk, wv, wo, q_ln_w, k_ln_w):
    x = np.asarray(hidden_states, dtype=np.float32)[0]        # [S, HID]
    xT = np.ascontiguousarray(x.T).astype(BF16NP)             # [HID, S]
    xP = np.ascontiguousarray(
        xT.reshape(NK, 128, 2, 1024).transpose(2, 0, 1, 3)
    ).reshape(2 * NK * 128, 1024)
    pos = np.asarray(position_ids)[0].astype(np.float32)      # [S]
    inv = 1.0 / (ROPE_THETA ** (np.arange(0, HD, 2, dtype=np.float32) / HD))
    ang = pos[:, None] * inv[None, :]                         # [S, 32]
    emb = np.concatenate([ang, ang], axis=1)                  # [S, 64]
    cosT = np.cos(emb).T.astype(np.float32)                   # [64, S]
    sinT = np.sin(emb).T.astype(np.float32)
    ss = sinT.copy()
    ss[0:32] = -sinT[0:32]
    cos2 = np.tile(cosT, (2, 1))
    ss2 = np.tile(ss, (2, 1))

    e2 = np.zeros((2, 128), dtype=np.float32)
    e2[0, 0:64] = 1.0
    e2[1, 64:128] = 1.0
    ew_q = np.zeros((2, 128), dtype=np.float32)
    ew_q[0, 0:64] = q_ln_w
    ew_q[1, 64:128] = q_ln_w
    ew_k = np.zeros((2, 128), dtype=np.float32)
    ew_k[1, 64:128] = k_ln_w
    msk = (np.arange(128)[:, None] <= np.arange(128)[None, :]) \
        .astype(np.float32)
    ident = np.eye(64, dtype=np.float32)
    r2 = np.zeros((128, 64), dtype=np.float32)
    for g in range(2):
        r2[64 * g + np.arange(64), np.arange(64)] = 1.0
    m_ = np.arange(128)
    pq_ = np.zeros((128, 128), dtype=np.float32)
    pq_[m_ ^ 32, m_] = 1.0
    pa_ = np.zeros((128, 128), dtype=np.float32)
    pa_[64 + (m_ & 63), m_] = 1.0
    pb_ = np.zeros((128, 128), dtype=np.float32)
    pb_[64 + ((m_ & 63) ^ 32), m_] = 1.0

    wq_ = np.asarray(wq, dtype=np.float32)
    wk_ = np.asarray(wk, dtype=np.float32)
    wv_ = np.asarray(wv, dtype=np.float32)
    wo_ = np.asarray(wo, dtype=np.float32)

    def pretile(w):  # [HID, N] -> [128, NK*N] ktile-blocked
        n = w.shape[1]
        return np.ascontiguousarray(
            w.reshape(NK, 128, n).transpose(1, 0, 2).reshape(128, NK * n))

    wof = pretile(wo_).astype(BF16NP)

    in_maps = []
    for c in range(N_CORES):
        qcols = slice(256 * c, 256 * (c + 1))
        kvcols = slice(64 * c, 64 * (c + 1))
        wq_c = np.ascontiguousarray(wq_[:, qcols])
        wkv_c = np.concatenate([wv_[:, kvcols], wk_[:, kvcols]], axis=1)
        in_maps.append({
            "xP": xP,
            "wq0": pretile(wq_c[:, 0:128]).astype(BF16NP),
            "wq1": pretile(wq_c[:, 128:256]).astype(BF16NP),
            "wkv": pretile(wkv_c).astype(BF16NP),
            "wof": wof,
            "cos2": cos2.astype(BF16NP),
            "ss2": ss2.astype(BF16NP),
            "ew_q": ew_q.astype(BF16NP),
            "ew_k": ew_k.astype(BF16NP),
            "e2": e2.astype(BF16NP),
            "e2t": np.ascontiguousarray(e2.T).astype(BF16NP),
            "mask": msk.astype(BF16NP),
            "ident": ident.astype(BF16NP),
            "r2": r2.astype(BF16NP),
            "pq_": pq_.astype(BF16NP),
            "pa_": pa_.astype(BF16NP),
            "pb_": pb_.astype(BF16NP),
        })
    return in_maps


def kernel(hidden_states, position_ids, wq, wk, wv, wo, q_ln_w, k_ln_w):
    global _NC_CACHE, LAST_RESULTS
    if _NC_CACHE is None:
        _NC_CACHE = _build()
    nc = _NC_CACHE
    in_maps = _host_prep(hidden_states, position_ids, wq, wk, wv, wo,
                         q_ln_w, k_ln_w)
    res = bass_utils.run_bass_kernel_spmd(
        nc, in_maps, core_ids=list(range(N_CORES)))
    LAST_RESULTS = res
    out = np.empty((S, HID), dtype=np.float32)
    for c in range(N_CORES):
        out[128 * c:128 * c + 128, :] = res.results[c]["out_rs"]
        out[1024 + 64 * c:1024 + 64 * c + 64, :] = \
            res.results[c]["out_r1T"].T
        out[1536 + 64 * c:1536 + 64 * c + 64, :] = \
            res.results[c]["out_r2T"].T
    return out.reshape(1, S, HID)


# revision 12
# speedup vs baseline: 1.0489x; 1.0139x over previous
"""GQA attention (B=1, S=2048, H=2048, 32 q-heads / 8 kv-heads, hd=64)
on 8 Trainium2 NeuronCores.

Sharding: tensor-parallel over heads for QKV+attention (core c owns
q-heads 4c..4c+3 and kv-head c), then sequence-parallel o_proj via
three AllToAll rounds aligned to q-chunk boundaries (q [0,1024) /
[1024,1536) / [1536,2048)). Per round, core c owns qn={128,64,64}
query rows; each core holds the FULL wo (bf16). Round 0's o_proj
matmuls are interleaved into qchunk 3's tile loop (its A2A completes
mid-attention); rounds 1/2 (M=64) use 2-way PE column tiling + a
transpose-reduce matmul, producing transposed outputs (out_r1T /
out_r2T) that the host fixes up.

Engine queues are FIFO in emission order, so long-dependency DMAs
must not sit ahead of staging DMAs: the rope half-swap and the kv
k-duplication are done as PE permutation matmuls (consts pq/pa/pb)
instead of SBUF-shift DMAs. B-phase activations are grouped (all Ln,
then all Exp) to avoid ACT table thrash against the attention Exp
stream; 1/l uses the fast DVE reciprocal. A tiny AllToAll prewarms
the collective path.
"""
import numpy as np
import sys

sys.path.insert(0, "/opt/trn_rl_repo")

import concourse.bacc as bacc  # noqa: E402
import concourse.mybir as mybir  # noqa: E402
import concourse.tile as tile  # noqa: E402
from concourse import bass_utils  # noqa: E402

f32 = mybir.dt.float32
bf16 = mybir.dt.bfloat16
AF = mybir.ActivationFunctionType
BF16NP = mybir.dt.np(bf16)

N_CORES = 8
S = 2048
HID = 2048
HD = 64
ROPE_THETA = 10000.0
RMS_EPS = 1e-6
SCALING = HD ** -0.5              # 0.125
NK = HID // 128                   # 16 contraction tiles
# rounds: (q_start, per-core qpos)
ROUNDS = ((0, 128), (1024, 64), (1536, 64))

_NC_CACHE = None
LAST_RESULTS = None


def _build():
    nc = bacc.Bacc("TRN2", target_bir_lowering=False, debug=False,
                   num_devices=N_CORES)

    def din(name, shape, dt):
        return nc.dram_tensor(name, shape, dt, kind="ExternalInput").ap()

    xP = din("xP", [2 * NK * 128, 1024], bf16)
    wq0 = din("wq0", [128, HID], bf16)
    wq1 = din("wq1", [128, HID], bf16)
    wkv = din("wkv", [128, HID], bf16)     # [wv | wk] columns pretiled
    wof = din("wof", [128, NK * HID], bf16)  # FULL wo, pretiled
    cos2 = din("cos2", [128, S], bf16)
    ss2 = din("ss2", [128, S], bf16)
    ew_q = din("ew_q", [2, 128], bf16)
    ew_k = din("ew_k", [2, 128], bf16)
    e2 = din("e2", [2, 128], bf16)
    e2t = din("e2t", [128, 2], bf16)
    mask = din("mask", [128, 128], bf16)
    ident = din("ident", [64, 64], bf16)
    r2 = din("r2", [128, 64], bf16)        # R2[64g+j, j] = 1
    pq_ = din("pq_", [128, 128], bf16)     # rope swap perm
    pa_ = din("pa_", [128, 128], bf16)     # kv nrm-dup perm
    pb_ = din("pb_", [128, 128], bf16)     # kv sh-dup perm

    out_rs = nc.dram_tensor("out_rs", [128, S], f32,
                            kind="ExternalOutput").ap()
    out_r1T = nc.dram_tensor("out_r1T", [HID, 64], f32,
                             kind="ExternalOutput").ap()
    out_r2T = nc.dram_tensor("out_r2T", [HID, 64], f32,
                             kind="ExternalOutput").ap()

    with tile.TileContext(nc) as tc:
        with tc.tile_pool(name="consts", bufs=1) as cp, \
             tc.tile_pool(name="dram", bufs=1, space="DRAM") as dp:
            c_wq0 = cp.tile([128, HID], bf16, tag="w")
            c_wq1 = cp.tile([128, HID], bf16, tag="w2")
            c_wkv = cp.tile([128, HID], bf16, tag="w3")
            c_wo = cp.tile([128, NK * HID], bf16, tag="w4")
            c_cos = cp.tile([128, S], bf16, tag="c1")
            c_ss = cp.tile([128, S], bf16, tag="c2")
            c_ewq = cp.tile([2, 128], bf16, tag="c3")
            c_ewk = cp.tile([2, 128], bf16, tag="c4")
            c_e2 = cp.tile([2, 128], bf16, tag="c5")
            c_e2t = cp.tile([128, 2], bf16, tag="c5t")
            c_mask = cp.tile([128, 128], bf16, tag="c6")
            c_id = cp.tile([64, 64], bf16, tag="c7")
            c_r2 = cp.tile([128, 64], bf16, tag="c7r")
            c_pq = cp.tile([128, 128], bf16, tag="cpq")
            c_pa = cp.tile([128, 128], bf16, tag="cpa")
            c_pb = cp.tile([128, 128], bf16, tag="cpb")
            c_eps = cp.tile([2, 1], f32, tag="c8")
            c_scr = cp.tile([128, 640], bf16, tag="c9")

            nc.vector.memset(c_scr[:], 0.0)
            nc.vector.memset(c_eps[:], RMS_EPS)

            pre_in = dp.tile([8, 16], bf16, name="prei", tag="pi")
            pre_out = dp.tile([8, 16], bf16, name="preo", tag="po")
            a2a_in = [dp.tile([S, qn], bf16, name=f"a2ai{r}",
                              tag=f"ai{r}")
                      for r, (_, qn) in enumerate(ROUNDS)]
            a2a_out = [dp.tile([S, qn], bf16, name=f"a2ao{r}",
                               tag=f"ao{r}")
                       for r, (_, qn) in enumerate(ROUNDS)]

            # prewarm the collective path with a tiny AllToAll
            nc.sync.dma_start(pre_in[:], c_scr[0:8, 0:16])
            nc.gpsimd.collective_compute(
                "AllToAll", mybir.AluOpType.bypass,
                replica_groups=[list(range(N_CORES))],
                ins=[pre_in[:, :].opt()], outs=[pre_out[:, :].opt()])

            # weight loads first (halves, so ktile 0-7 weights land
            # early); small consts after the first halves
            nc.sync.dma_start(c_wq0[:, 0:1024], wq0[:, 0:1024])
            nc.scalar.dma_start(c_wq1[:, 0:1024], wq1[:, 0:1024])
            nc.gpsimd.dma_start(c_wkv[:, 0:1024], wkv[:, 0:1024])
            nc.sync.dma_start(c_wq0[:, 1024:2048], wq0[:, 1024:2048])
            nc.scalar.dma_start(c_wq1[:, 1024:2048], wq1[:, 1024:2048])
            nc.gpsimd.dma_start(c_wkv[:, 1024:2048], wkv[:, 1024:2048])
            nc.scalar.dma_start(c_e2t[:], e2t)
            nc.scalar.dma_start(c_ewq[:], ew_q)
            nc.scalar.dma_start(c_ewk[:], ew_k)
            nc.scalar.dma_start(c_id[:], ident)
            nc.scalar.dma_start(c_e2[:], e2)
            nc.scalar.dma_start(c_r2[:], r2)
            nc.scalar.dma_start(c_mask[:], mask)
            nc.scalar.dma_start(c_pq[:], pq_)
            nc.scalar.dma_start(c_pa[:], pa_)
            nc.scalar.dma_start(c_pb[:], pb_)

            qkv = {
                "q0": cp.tile([128, S], bf16, tag="q0", name="q0"),
                "q1": cp.tile([128, S], bf16, tag="q1", name="q1"),
                "kv": cp.tile([128, S], bf16, tag="kv", name="kv"),
            }
            qr0h = [cp.tile([128, 1024], bf16, tag=f"qr0{h}",
                            name=f"qr0{h}") for h in range(2)]
            qr1h = [cp.tile([128, 1024], bf16, tag=f"qr1{h}",
                            name=f"qr1{h}") for h in range(2)]
            krdh = [cp.tile([128, 1024], bf16, tag=f"krd{h}",
                            name=f"krd{h}") for h in range(2)]
            vah = [cp.tile([128, 8 * (HD + 1)], bf16, tag=f"va{h}",
                           name=f"va{h}") for h in range(2)]
            attn_bf = [cp.tile([128, S], bf16, tag=f"abf{i}",
                               name=f"abf{i}") for i in range(2)]

            with tc.tile_pool(name="xt", bufs=4) as xp, \
                 tc.tile_pool(name="sbB", bufs=2) as sbB:

                def phase_a(qh, psA):
                    hs = slice(1024 * qh, 1024 * qh + 1024)
                    pq = [psA.tile([128, 1024], f32, tag="pa",
                                   name=f"pa{qh}_{j}") for j in range(3)]
                    for t in range(NK):
                        xt = xp.tile([128, 1024], bf16, tag="xt")
                        eng = (nc.sync, nc.scalar, nc.gpsimd)[t % 3]
                        xr = (qh * NK + t) * 128
                        eng.dma_start(xt[:], xP[xr:xr + 128, :])
                        st = (t == 0)
                        sp = (t == NK - 1)
                        tc_ = slice(128 * t, 128 * (t + 1))
                        for j, w in ((0, c_wq0), (1, c_wq1), (2, c_wkv)):
                            nc.tensor.matmul(pq[j][:, 0:512], w[:, tc_],
                                             xt[:, 0:512],
                                             start=st, stop=sp)
                            nc.tensor.matmul(pq[j][:, 512:1024],
                                             w[:, tc_], xt[:, 512:1024],
                                             start=st, stop=sp)
                    for j, key in ((0, "q0"), (1, "q1"), (2, "kv")):
                        nc.vector.tensor_copy(qkv[key][:, hs], pq[j][:])

                # --- B-phase, split so ACT ops group by table-set ---
                def b_p1a(qh, si, key, is_kv, psP, ptag):
                    hs = slice(1024 * qh, 1024 * qh + 1024)
                    src = qkv[key]
                    if is_kv:
                        nc.gpsimd.memset(vah[qh][:], 1.0)
                        for lt in range(8):
                            ptr = psP.tile([128, 64], bf16, tag=ptag,
                                           name=f"ptr{qh}_{lt}")
                            nc.tensor.transpose(
                                ptr[:],
                                src[0:64, 1024 * qh + 128 * lt:
                                    1024 * qh + 128 * (lt + 1)],
                                c_id[:])
                            nc.vector.tensor_copy(
                                vah[qh][:, (HD + 1) * lt:
                                        (HD + 1) * lt + HD],
                                ptr[:])
                    sq = sbB.tile([128, 1024], bf16, tag="sq",
                                  bufs=2, name=f"sq{qh}_{si}")
                    nc.vector.tensor_mul(sq[:], src[:, hs], src[:, hs])
                    lnvs = {}
                    for u in range(2):
                        us = slice(512 * u, 512 * u + 512)
                        pss = psP.tile([2, 512], f32, tag=ptag,
                                       name=f"ss{qh}_{si}_{u}")
                        nc.tensor.matmul(pss[:], c_e2t[:], sq[:, us],
                                         start=True, stop=True)
                        lnv = sbB.tile([2, 512], bf16, tag="lnv",
                                       bufs=6, name=f"lnv{qh}{si}{u}")
                        nc.scalar.activation(lnv[:], pss[:], AF.Ln,
                                             scale=1.0 / HD,
                                             bias=c_eps[:])
                        lnvs[u] = lnv
                    return lnvs

                def b_p1b(qh, si, lnvs):
                    rstds = {}
                    for u in range(2):
                        rr = sbB.tile([2, 512], bf16, tag="rstdr",
                                      bufs=6, name=f"rr{qh}{si}{u}")
                        nc.scalar.activation(rr[:], lnvs[u][:],
                                             AF.Exp, scale=-0.5)
                        rstds[u] = rr
                    return rstds

                def b_p2(qh, si, key, ew, dst, is_kv, rstds, psP, ptag):
                    src = qkv[key]
                    for u in range(2):
                        cs = slice(1024 * qh + 512 * u,
                                   1024 * qh + 512 * u + 512)
                        us = slice(512 * u, 512 * u + 512)
                        pb = psP.tile([128, 512], f32, tag=ptag,
                                      name=f"pb{qh}_{si}_{u}")
                        nc.tensor.matmul(pb[:], ew[:], rstds[u][:],
                                         start=True, stop=True)
                        nrm = sbB.tile([128, 512], bf16, tag="nrm",
                                       bufs=4, name=f"nrm{qh}{si}{u}")
                        nc.vector.tensor_mul(nrm[:], src[:, cs], pb[:])
                        if is_kv:
                            pA = psP.tile([128, 512], f32, tag=ptag,
                                          name=f"pA{qh}{si}{u}")
                            nc.tensor.matmul(pA[:], c_pa[:], nrm[:],
                                             start=True, stop=True)
                            pB = psP.tile([128, 512], f32, tag=ptag,
                                          name=f"pB{qh}{si}{u}")
                            nc.tensor.matmul(pB[:], c_pb[:], nrm[:],
                                             start=True, stop=True)
                            srcA, srcB = pA, pB
                        else:
                            pS = psP.tile([128, 512], f32, tag=ptag,
                                          name=f"pS{qh}{si}{u}")
                            nc.tensor.matmul(pS[:], c_pq[:], nrm[:],
                                             start=True, stop=True)
                            srcA, srcB = nrm, pS
                        t2 = sbB.tile([128, 512], f32, tag="t2",
                                      bufs=2, name=f"t2{qh}{si}{u}")
                        nc.vector.tensor_mul(t2[:], srcB[:],
                                             c_ss[:, cs])
                        t1 = sbB.tile([128, 512], f32, tag="sh",
                                      bufs=2, name=f"t1{qh}{si}{u}")
                        nc.vector.tensor_mul(t1[:], srcA[:],
                                             c_cos[:, cs])
                        nc.vector.tensor_add(dst[:, us], t1[:], t2[:])

                B0 = (("kv", c_ewk, True), ("q0", c_ewq, False),
                      ("q1", c_ewq, False))

                def b_group(qh, dsts, psP, ptag):
                    ls = [b_p1a(qh, si, key, ik, psP, ptag)
                          for si, (key, _, ik) in enumerate(B0)]
                    rs = [b_p1b(qh, si, ls[si]) for si in range(3)]
                    for si, (key, ew, ik) in enumerate(B0):
                        b_p2(qh, si, key, ew, dsts[si], ik, rs[si],
                             psP, ptag)

                # ---- scope 1: warmup + A0 + B0-group + A1 ----
                with tc.tile_pool(name="psA", bufs=3,
                                  space="PSUM") as psA, \
                     tc.tile_pool(name="psM", bufs=2,
                                  space="PSUM") as psM:
                    pwm = psM.tile([128, 512], f32, tag="m", name="pwm")
                    for i in range(32):
                        nc.tensor.matmul(pwm[:], c_scr[:, 0:128],
                                         c_scr[:, 128:640],
                                         start=True, stop=True)
                    phase_a(0, psA)
                    nc.gpsimd.dma_start(c_cos[:], cos2)
                    nc.gpsimd.dma_start(c_ss[:], ss2)
                    b_group(0, (krdh[0], qr0h[0], qr1h[0]), psM, "m")
                    phase_a(1, psA)
                    for h in range(8):
                        cs_ = slice(4096 * h, 4096 * (h + 1))
                        nc.gpsimd.dma_start(c_wo[:, cs_], wof[:, cs_])

                # ---- scope 2: B1 + qchunks + A2As + o_proj ----
                with tc.tile_pool(name="sbC", bufs=4) as sbC, \
                     tc.tile_pool(name="atk", bufs=2) as akp, \
                     tc.tile_pool(name="psS", bufs=2,
                                  space="PSUM") as psS, \
                     tc.tile_pool(name="psPV", bufs=2,
                                  space="PSUM") as psPV, \
                     tc.tile_pool(name="psB", bufs=1,
                                  space="PSUM") as psB, \
                     tc.tile_pool(name="psO", bufs=1,
                                  space="PSUM") as psO:

                    def qchunk(q0, W, inter=None,
                               istart=(0, 10), irate=7):
                        qs = slice(q0, q0 + W)
                        qhh = q0 // 1024
                        qcol0 = q0 - 1024 * qhh
                        ntile = (q0 + W) // 128
                        t0k = q0 // 128
                        rnd = 0 if q0 < 1024 else (1 if q0 < 1536
                                                   else 2)
                        rs_, qn = ROUNDS[rnd]
                        for hp, qrh in ((0, qr0h), (1, qr1h)):
                            qr = qrh[qhh]
                            ppv_a = psPV.tile([65, W], f32, tag="pv",
                                              name=f"pva{q0}_{hp}")
                            ppv_b = psPV.tile([65, W], f32, tag="pv",
                                              name=f"pvb{q0}_{hp}")

                            def emit_pv(p):
                                pt, off, qlen, vs, va, st, sp = p
                                nc.tensor.matmul(
                                    ppv_a[:, off:W], va[:, vs],
                                    pt[:, 0:qlen], start=st, stop=sp)
                                nc.tensor.matmul(
                                    ppv_b[:, off:W], va[:, vs],
                                    pt[:, 512:512 + qlen],
                                    start=st, stop=sp)

                            pvq = []
                            for t in range(ntile):
                                r = t - t0k
                                off = max(0, r) * 128
                                qlo = qcol0 + off
                                qlen = W - off
                                kh = t // 8
                                krd = krdh[kh]
                                v_aug = vah[kh]
                                tl = t - 8 * kh
                                kc = slice(128 * tl, 128 * (tl + 1))
                                vs = slice((HD + 1) * tl,
                                           (HD + 1) * tl + HD + 1)
                                st = (t == 0)
                                sp = (t == ntile - 1)
                                ps_s = psS.tile([128, 1024], f32,
                                                tag="s")
                                nc.tensor.matmul(
                                    ps_s[:, 0:qlen], krd[0:64, kc],
                                    qr[0:64, qlo:qlo + qlen],
                                    start=True, stop=True)
                                nc.tensor.matmul(
                                    ps_s[:, 512:512 + qlen],
                                    krd[64:128, kc],
                                    qr[64:128, qlo:qlo + qlen],
                                    start=True, stop=True)
                                pt = sbC.tile([128, 1024], bf16,
                                              tag="pt")
                                nc.scalar.activation(
                                    pt[:, 0:512 + qlen],
                                    ps_s[:, 0:512 + qlen],
                                    AF.Exp, scale=SCALING)
                                if r >= 0:
                                    nc.vector.tensor_mul(
                                        pt[:, 0:128], pt[:, 0:128],
                                        c_mask[:])
                                    nc.vector.tensor_mul(
                                        pt[:, 512:640], pt[:, 512:640],
                                        c_mask[:])
                                pvq.append((pt, off, qlen, vs,
                                            v_aug, st, sp))
                                if len(pvq) > 1:
                                    emit_pv(pvq.pop(0))
                                if inter is not None and \
                                        (hp, t) >= istart:
                                    inter(irate)
                            emit_pv(pvq.pop(0))
                            # normalize + stage
                            nc.vector.tensor_copy(
                                attn_bf[hp][0:64, qs], ppv_a[0:64, :])
                            stgb = sbC.tile([64, W], bf16, tag="stg",
                                            bufs=2,
                                            name=f"sg{q0}_{hp}")
                            nc.vector.tensor_copy(stgb[:],
                                                  ppv_b[0:64, :])
                            nc.sync.dma_start(
                                attn_bf[hp][64:128, qs], stgb[:])
                            la = sbC.tile([65, W], f32, tag="la",
                                          bufs=2, name=f"la{q0}_{hp}")
                            nc.vector.tensor_copy(la[64:65, :],
                                                  ppv_a[64:65, :])
                            lb = sbC.tile([65, W], f32, tag="la",
                                          bufs=2, name=f"lb{q0}_{hp}")
                            nc.vector.tensor_copy(lb[64:65, :],
                                                  ppv_b[64:65, :])
                            lf = sbC.tile([2, W], f32, tag="lf",
                                          bufs=2, name=f"lf{q0}_{hp}")
                            nc.sync.dma_start(lf[0:1, :], la[64:65, :])
                            nc.gpsimd.dma_start(lf[1:2, :],
                                                lb[64:65, :])
                            rlf = sbC.tile([2, W], f32, tag="lf",
                                           bufs=2, name=f"rf{q0}_{hp}")
                            nc.vector.reciprocal_approx_fast(rlf[:],
                                                             lf[:])
                            rlb = sbC.tile([2, W], bf16, tag="rlb",
                                           bufs=2, name=f"rb{q0}_{hp}")
                            nc.vector.tensor_copy(rlb[:], rlf[:])
                            pb = psB.tile([128, W], f32, tag="b",
                                          name=f"qpb{q0}_{hp}")
                            nc.tensor.matmul(pb[:], c_e2[:], rlb[:],
                                             start=True, stop=True)
                            nc.vector.tensor_mul(
                                attn_bf[hp][:, qs],
                                attn_bf[hp][:, qs], pb[:])
                            for c in range(N_CORES):
                                a = max(q0, rs_ + qn * c)
                                b = min(q0 + W, rs_ + qn * (c + 1))
                                if a < b:
                                    seng = (nc.gpsimd, nc.sync,
                                            nc.scalar)[c % 3]
                                    rr_ = 256 * c + 128 * hp
                                    seng.dma_start(
                                        a2a_in[rnd][rr_:rr_ + 128,
                                                    a - rs_ - qn * c:
                                                    b - rs_ - qn * c],
                                        attn_bf[hp][:, a:b])

                    def a2a(rnd):
                        nc.gpsimd.collective_compute(
                            "AllToAll",
                            mybir.AluOpType.bypass,
                            replica_groups=[list(range(N_CORES))],
                            ins=[a2a_in[rnd][:, :].opt()],
                            outs=[a2a_out[rnd][:, :].opt()],
                        )

                    def oproj_tail(rnd):
                        outT = out_r1T if rnd == 1 else out_r2T
                        attk = akp.tile([128, NK * 64], bf16,
                                        tag="atk", name=f"atk{rnd}")
                        for kk in range(NK):
                            eng = (nc.sync, nc.scalar,
                                   nc.gpsimd)[kk % 3]
                            eng.dma_start(
                                attk[:, 64 * kk:64 * (kk + 1)],
                                a2a_out[rnd][128 * kk:
                                             128 * (kk + 1), :])
                        pos = psO.tile([128, 512], f32, tag="o",
                                       name=f"pos{rnd}")

                        def flush(p):
                            sub, wcs, ost4 = p
                            pr = psB.tile([128, 128], f32, tag="b",
                                          name=f"pr{rnd}_{sub}")
                            nc.tensor.matmul(pr[:, 0:64],
                                             ost4[:, 0:128], c_r2[:],
                                             start=True, stop=True)
                            nc.tensor.matmul(pr[:, 64:128],
                                             ost4[:, 128:256],
                                             c_r2[:],
                                             start=True, stop=True)
                            o2 = sbC.tile([128, 128], f32, tag="ost",
                                          bufs=4,
                                          name=f"o2_{rnd}_{sub}")
                            nc.scalar.activation(o2[:], pr[:], AF.Copy)
                            e1 = nc.sync if sub % 2 == 0 else nc.scalar
                            e2_ = nc.scalar if sub % 2 == 0 else \
                                nc.sync
                            e1.dma_start(outT[wcs:wcs + 128, :],
                                         o2[:, 0:64])
                            e2_.dma_start(outT[wcs + 128:wcs + 256, :],
                                          o2[:, 64:128])

                        pend = None
                        for sub in range(8):
                            h = 256 * (sub % 2)
                            wcs = 256 * sub
                            for i in range(8):
                                for g in range(2):
                                    kk = 8 * g + i
                                    nc.tensor.matmul(
                                        pos[64 * g:64 * g + 64,
                                            h:h + 256],
                                        attk[:, 64 * kk:64 * kk + 64],
                                        c_wo[:, HID * kk + wcs:
                                             HID * kk + wcs + 256],
                                        start=(i == 0), stop=(i == 7),
                                        tile_position=(0, 64 * g))
                            ost4 = sbC.tile([128, 256], bf16,
                                            tag="ost", bufs=4,
                                            name=f"o4_{rnd}_{sub}")
                            nc.scalar.activation(ost4[:],
                                                 pos[:, h:h + 256],
                                                 AF.Copy)
                            if pend is not None:
                                flush(pend)
                            pend = (sub, wcs, ost4)
                        flush(pend)

                    qchunk(0, 512)
                    b_group(1, (krdh[1], qr0h[1], qr1h[1]), psS, "s")
                    qchunk(512, 512)
                    a2a(0)
                    # prefetch round-0 attk on the (idle) scalar queue
                    attk0 = akp.tile([128, NK * 128], bf16, tag="atk",
                                     name="atk0")
                    for kk in range(NK):
                        nc.scalar.dma_start(
                            attk0[:, 128 * kk:128 * (kk + 1)],
                            a2a_out[0][128 * kk:128 * (kk + 1), :])
                    pos0 = psO.tile([128, 512], f32, tag="o",
                                    name="pos0")

                    # round-0 o_proj emission steps, interleaved
                    # into qchunk(1536)'s tile loop
                    def op0_steps():
                        for sub in range(8):
                            h = 256 * (sub % 2)
                            wcs = 256 * sub
                            for kk in range(NK):
                                yield ("mm", sub, h, wcs, kk)
                            yield ("fin", sub, h, wcs, 0)
                    _op0 = op0_steps()
                    _done = [False]

                    def op0_drain(n):
                        for _ in range(n):
                            step = next(_op0, None)
                            if step is None:
                                _done[0] = True
                                return
                            kind, sub, h, wcs, kk = step
                            if kind == "mm":
                                nc.tensor.matmul(
                                    pos0[:, h:h + 256],
                                    attk0[:, 128 * kk:128 * (kk + 1)],
                                    c_wo[:, HID * kk + wcs:
                                         HID * kk + wcs + 256],
                                    start=(kk == 0),
                                    stop=(kk == NK - 1))
                            else:
                                ost = sbC.tile([128, 256], f32,
                                               tag="ost", bufs=4,
                                               name=f"ost0_{sub}")
                                nc.vector.tensor_copy(
                                    ost[:], pos0[:, h:h + 256])
                                nc.scalar.dma_start(
                                    out_rs[:, wcs:wcs + 256], ost[:])

                    qchunk(1024, 512, inter=op0_drain,
                           istart=(1, 4), irate=3)
                    a2a(1)
                    qchunk(1536, 512, inter=op0_drain,
                           istart=(0, 0), irate=3)
                    while not _done[0]:
                        op0_drain(8)
                    a2a(2)
                    oproj_tail(1)
                    oproj_tail(2)

    nc.compile()
    return nc


def _host_prep(hidden_states, position_ids, wq, wk, wv, wo, q_ln_w, k_ln_w):
    x = np.asarray(hidden_states, dtype=np.float32)[0]        # [S, HID]
    xT = np.ascontiguousarray(x.T).astype(BF16NP)             # [HID, S]
    xP = np.ascontiguousarray(
        xT.reshape(NK, 128, 2, 1024).transpose(2, 0, 1, 3)
    ).reshape(2 * NK * 128, 1024)
    pos = np.asarray(position_ids)[0].astype(np.float32)      # [S]
    inv = 1.0 / (ROPE_THETA ** (np.arange(0, HD, 2, dtype=np.float32) / HD))
    ang = pos[:, None] * inv[None, :]                         # [S, 32]
    emb = np.concatenate([ang, ang], axis=1)                  # [S, 64]
    cosT = np.cos(emb).T.astype(np.float32)                   # [64, S]
    sinT = np.sin(emb).T.astype(np.float32)
    ss = sinT.copy()
    ss[0:32] = -sinT[0:32]
    cos2 = np.tile(cosT, (2, 1))
    ss2 = np.tile(ss, (2, 1))

    e2 = np.zeros((2, 128), dtype=np.float32)
    e2[0, 0:64] = 1.0
    e2[1, 64:128] = 1.0
    ew_q = np.zeros((2, 128), dtype=np.float32)
    ew_q[0, 0:64] = q_ln_w
    ew_q[1, 64:128] = q_ln_w
    ew_k = np.zeros((2, 128), dtype=np.float32)
    ew_k[1, 64:128] = k_ln_w
    msk = (np.arange(128)[:, None] <= np.arange(128)[None, :]) \
        .astype(np.float32)
    ident = np.eye(64, dtype=np.float32)
    r2 = np.zeros((128, 64), dtype=np.float32)
    for g in range(2):
        r2[64 * g + np.arange(64), np.arange(64)] = 1.0
    m_ = np.arange(128)
    pq_ = np.zeros((128, 128), dtype=np.float32)
    pq_[m_ ^ 32, m_] = 1.0
    pa_ = np.zeros((128, 128), dtype=np.float32)
    pa_[64 + (m_ & 63), m_] = 1.0
    pb_ = np.zeros((128, 128), dtype=np.float32)
    pb_[64 + ((m_ & 63) ^ 32), m_] = 1.0

    wq_ = np.asarray(wq, dtype=np.float32)
    wk_ = np.asarray(wk, dtype=np.float32)
    wv_ = np.asarray(wv, dtype=np.float32)
    wo_ = np.asarray(wo, dtype=np.float32)

    def pretile(w):  # [HID, N] -> [128, NK*N] ktile-blocked
        n = w.shape[1]
        return np.ascontiguousarray(
            w.reshape(NK, 128, n).transpose(1, 0, 2).reshape(128, NK * n))

    wof = pretile(wo_).astype(BF16NP)

    in_maps = []
    for c in range(N_CORES):
        qcols = slice(256 * c, 256 * (c + 1))
        kvcols = slice(64 * c, 64 * (c + 1))
        wq_c = np.ascontiguousarray(wq_[:, qcols])
        wkv_c = np.concatenate([wv_[:, kvcols], wk_[:, kvcols]], axis=1)
        in_maps.append({
            "xP": xP,
            "wq0": pretile(wq_c[:, 0:128]).astype(BF16NP),
            "wq1": pretile(wq_c[:, 128:256]).astype(BF16NP),
            "wkv": pretile(wkv_c).astype(BF16NP),
            "wof": wof,
            "cos2": cos2.astype(BF16NP),
            "ss2": ss2.astype(BF16NP),
            "ew_q": ew_q.astype(BF16NP),
            "ew_k": ew_k.astype(BF16NP),
            "e2": e2.astype(BF16NP),
            "e2t": np.ascontiguousarray(e2.T).astype(BF16NP),
            "mask": msk.astype(BF16NP),
            "ident": ident.astype(BF16NP),
            "r2": r2.astype(BF16NP),
            "pq_": pq_.astype(BF16NP),
            "pa_": pa_.astype(BF16NP),
            "pb_": pb_.astype(BF16NP),
        })
    return in_maps


def kernel(hidden_states, position_ids, wq, wk, wv, wo, q_ln_w, k_ln_w):
    global _NC_CACHE, LAST_RESULTS
    if _NC_CACHE is None:
        _NC_CACHE = _build()
    nc = _NC_CACHE
    in_maps = _host_prep(hidden_states, position_ids, wq, wk, wv, wo,
                         q_ln_w, k_ln_w)
    res = bass_utils.run_bass_kernel_spmd(
        nc, in_maps, core_ids=list(range(N_CORES)))
    LAST_RESULTS = res
    out = np.empty((S, HID), dtype=np.float32)
    for c in range(N_CORES):
        out[128 * c:128 * c + 128, :] = res.results[c]["out_rs"]
        out[1024 + 64 * c:1024 + 64 * c + 64, :] = \
            res.results[c]["out_r1T"].T
        out[1536 + 64 * c:1536 + 64 * c + 64, :] = \
            res.results[c]["out_r2T"].T
    return out.reshape(1, S, HID)
